# revision 1
# baseline (speedup 1.0000x reference)
"""GraphMAE-style 2-layer GIN loss kernel for one TRN2 chip (8 NeuronCores).

Self-contained: builds + compiles a Bass/Tile SPMD kernel, shards the graph on
the host (dst-partitioned nodes + incident edges), runs on 8 cores via
run_bass_kernel_spmd, returns the scalar loss.

Design:
  - nodes sharded: core c owns rows [12500c, 12500(c+1)), padded to 12544
  - segment_sum = gather (batched indirect DMA from replicated bf16 tables)
    + one-hot matmul scatter into per-128-node-window PSUM accumulators
  - masked encoder input handled with a masked one-hot (S_x) + rank-1
    mask_token x count correction matmul
  - feat-major activations [128 feat, nodes]; BN stats = free-dim reduces,
    all-reduced across cores; BN affine + relu fused on ScalarE
  - h tables allgathered (bf16) between layers; final loss via small AllReduce
"""
import sys, os, contextlib
sys.path.insert(0, '/opt/trn_rl_repo')
import numpy as np
import ml_dtypes

from concourse import bass, bacc, tile, mybir
from concourse.bass_utils import run_bass_kernel_spmd

dt = mybir.dt
bf16 = ml_dtypes.bfloat16

# ---------------- problem constants ----------------
N = 100000
E = 800000
D = 128
H = 256
L = 2
NCORES = 8
NSHARD = N // NCORES          # 12500
NLOC = 12544                  # padded to 98*128
WW = 256                      # dst-window width (nodes)
WIN = NLOC // WW              # windows per core
NBLK = NLOC // 128            # 128-col blocks per core
BN_EPS = 1e-5
GB = 1                        # (vestigial) tile padding multiple
BLK_GRP = 4                   # node blocks per matmul group (free dim 512)


# ---------------- host-side graph preprocessing ----------------
def preprocess(src, dst, mask_nodes):
    """Build per-core edge streams + shared schedule."""
    src = np.asarray(src).astype(np.int64)
    dst = np.asarray(dst).astype(np.int64)
    core_of = dst // NSHARD
    dst_loc = dst % NSHARD
    win = dst_loc // WW
    dloc = dst_loc % WW

    is_masked = np.zeros(N, np.bool_)
    is_masked[np.asarray(mask_nodes, np.int64)] = True
    m_src = is_masked[src]

    per_core = []
    counts = np.zeros((NCORES, WIN), np.int64)
    for c in range(NCORES):
        sel = np.nonzero(core_of == c)[0]
        # sort by (window, src) - window order for PSUM schedule, src order for
        # HBM row-buffer locality within a window
        order = np.lexsort((src[sel], win[sel]))
        sel = sel[order]
        per_core.append(sel)
        cw = np.bincount(win[sel], minlength=WIN)
        counts[c] = cw
    tiles_per_win = np.maximum(1, (counts.max(axis=0) + 127) // 128)  # shared schedule
    T = int(tiles_per_win.sum())
    Tpad = ((T + GB - 1) // GB) * GB
    tiles_per_win[WIN - 1] += Tpad - T
    T = Tpad

    win_of_tile = np.repeat(np.arange(WIN), tiles_per_win)

    src0 = np.zeros((NCORES, T * 128), np.int32)
    src1 = np.zeros((NCORES, T * 128), np.int32)
    dstA = np.full((NCORES, T * 128), -1.0, np.float32)
    dstB = np.full((NCORES, T * 128), -1.0, np.float32)
    mflg = np.zeros((NCORES, T * 128), np.float32)
    tile_start = np.concatenate([[0], np.cumsum(tiles_per_win)]) * 128
    for c in range(NCORES):
        sel = per_core[c]
        wsel = win[sel]
        # place each window's edges at its tile range start
        ofs = np.concatenate([[0], np.cumsum(counts[c])])
        pos = tile_start[wsel] + (np.arange(len(sel)) - ofs[wsel])
        s = src[sel]
        src0[c, pos] = s
        src1[c, pos] = (s // NSHARD) * NLOC + (s % NSHARD)
        dstA[c, pos] = dloc[sel]
        mflg[c, pos] = m_src[sel]
        db = dstA[c].copy()
        db[pos[m_src[sel]]] = -1.0
        dstB[c] = db
    # reshape to [128, T] tile-major: edge (t, p) at stream position t*128+p
    def to_tiles(a, dtype):
        return np.ascontiguousarray(a.reshape(NCORES, T, 128).transpose(0, 2, 1)).astype(dtype)
    return dict(
        T=T, win_of_tile=win_of_tile,
        src0=to_tiles(src0, np.int32), src1=to_tiles(src1, np.int32),
        dstA=to_tiles(dstA, bf16), dstB=to_tiles(dstB, bf16),
        mflg=to_tiles(mflg, ml_dtypes.float8_e4m3),
    )


# ---------------- kernel builder ----------------
def build(T, win_of_tile, Mn):
    nc = bacc.Bacc("TRN2", target_bir_lowering=False, debug=False, num_devices=NCORES)
    rg = [list(range(NCORES))]

    def inp(name, shape, d):
        return nc.dram_tensor(name, shape, d, kind="ExternalInput")

    feat_tab = inp("feat_tab", [N, D], dt.float8e4)
    xT_own = inp("xT_own", [128, NLOC], dt.bfloat16)
    fT_own = inp("fT_own", [128, NLOC], dt.bfloat16)
    src0 = inp("src0", [128, T], dt.int32)
    src1 = inp("src1", [128, T], dt.int32)
    dstA = inp("dstA", [128, T], dt.bfloat16)
    dstB = inp("dstB", [128, T], dt.bfloat16)
    mflg = inp("mflg", [128, T], dt.float8e4)
    iota_c = inp("iota_c", [128, WW], dt.bfloat16)
    ones_c = inp("ones_c", [128, 1], dt.bfloat16)
    ident_c = inp("ident_c", [128, 128], dt.bfloat16)
    ttok_c = inp("ttok_c", [1, 128], dt.bfloat16)
    w_node = inp("w_node", [128, NBLK], dt.float32)
    W1 = {}; W2 = {}; G1 = {}; B1 = {}; G2 = {}; B2 = {}
    for e in range(2):
        for l in range(2):
            W1[e, l] = inp(f"w1_{e}{l}", [128, H], dt.bfloat16)
            W2[e, l] = inp(f"w2_{e}{l}", [H, 128], dt.bfloat16)
            G1[e, l] = inp(f"g1_{e}{l}", [128, 2], dt.float32)
            B1[e, l] = inp(f"b1_{e}{l}", [128, 2], dt.float32)
            G2[e, l] = inp(f"g2_{e}{l}", [128, 1], dt.float32)
            B2[e, l] = inp(f"b2_{e}{l}", [128, 1], dt.float32)
    out_loss = nc.dram_tensor("loss", [1, 1], dt.float32, kind="ExternalOutput")
    DEBUG = bool(int(os.environ.get("GIN_DEBUG", "0")))
    dbg = {}
    if DEBUG:
        for nm in ["zt0", "zt1", "h0", "h1", "zl0", "zl1", "hf0", "hf1"]:
            dbg[nm] = nc.dram_tensor(f"dbg_{nm}", [128, NLOC], dt.bfloat16, kind="ExternalOutput")
        dbg["st"] = nc.dram_tensor("dbg_st", [128, 8], dt.float32, kind="ExternalOutput")
        dbg["pools"] = nc.dram_tensor("dbg_pools", [128, 5], dt.float32, kind="ExternalOutput")

    ar_i = [0]  # AR sequence counter

    NBAT = T // GB
    first_of_win = {}
    last_of_win = {}
    for t in range(T):
        w = int(win_of_tile[t])
        if w not in first_of_win:
            first_of_win[w] = t
        last_of_win[w] = t

    with tile.TileContext(nc) as tc:
        with tc.tile_pool(name="const", bufs=1) as cst, \
             tc.tile_pool(name="streams", bufs=1) as stp, \
             tc.tile_pool(name="big", bufs=1) as bigp, \
             tc.tile_pool(name="gath", bufs=8) as gp, \
             tc.tile_pool(name="sti", bufs=4) as sti, \
             tc.tile_pool(name="small", bufs=4) as smp, \
             tc.tile_pool(name="stats", bufs=1) as statp, \
             tc.tile_pool(name="dram", bufs=2, space="DRAM") as dramp:

            htab_loc = dramp.tile([NLOC, 2 * D], dt.float8e4, tag="htab_loc", name="htab_loc")
            htab = dramp.tile([NLOC * NCORES, 2 * D], dt.float8e4, tag="htab", name="htab",
                              addr_space="Shared")

            iota_t = cst.tile([128, WW], dt.bfloat16)
            ones_t = cst.tile([128, 1], dt.bfloat16)
            ident_t = cst.tile([128, 128], dt.bfloat16)
            ttok_t = cst.tile([1, 128], dt.bfloat16)
            wnode_t = cst.tile([128, NBLK], dt.float32)
            nc.sync.dma_start(out=iota_t[:], in_=iota_c[:])
            nc.sync.dma_start(out=ones_t[:], in_=ones_c[:])
            nc.sync.dma_start(out=ident_t[:], in_=ident_c[:])
            nc.sync.dma_start(out=ttok_t[:], in_=ttok_c[:])
            nc.sync.dma_start(out=wnode_t[:], in_=w_node[:])
            wt = {}
            for e in range(2):
                for l in range(2):
                    wt[e, l] = dict(
                        w1=cst.tile([128, H], dt.bfloat16, tag=f"w1_{e}{l}", name=f"w1t{e}{l}"),
                        w2a=cst.tile([128, 128], dt.bfloat16, tag=f"w2a_{e}{l}", name=f"w2at{e}{l}"),
                        w2b=cst.tile([128, 128], dt.bfloat16, tag=f"w2b_{e}{l}", name=f"w2bt{e}{l}"),
                        g1=cst.tile([128, 2], dt.float32, tag=f"g1_{e}{l}", name=f"g1t{e}{l}"),
                        b1=cst.tile([128, 2], dt.float32, tag=f"b1_{e}{l}", name=f"b1t{e}{l}"),
                        g2=cst.tile([128, 1], dt.float32, tag=f"g2_{e}{l}", name=f"g2t{e}{l}"),
                        b2=cst.tile([128, 1], dt.float32, tag=f"b2_{e}{l}", name=f"b2t{e}{l}"),
                    )
                    nc.sync.dma_start(out=wt[e, l]["w1"][:], in_=W1[e, l][:])
                    nc.sync.dma_start(out=wt[e, l]["w2a"][:], in_=W2[e, l][:128, :])
                    nc.sync.dma_start(out=wt[e, l]["w2b"][:], in_=W2[e, l][128:, :])
                    nc.sync.dma_start(out=wt[e, l]["g1"][:], in_=G1[e, l][:])
                    nc.sync.dma_start(out=wt[e, l]["b1"][:], in_=B1[e, l][:])
                    nc.sync.dma_start(out=wt[e, l]["g2"][:], in_=G2[e, l][:])
                    nc.sync.dma_start(out=wt[e, l]["b2"][:], in_=B2[e, l][:])

            # edge streams resident
            src_sb = [stp.tile([128, T], dt.int32, tag=f"src{i}", name=f"src_sb{i}") for i in range(2)]
            nc.sync.dma_start(out=src_sb[0][:], in_=src0[:])
            nc.sync.dma_start(out=src_sb[1][:], in_=src1[:])
            dstA_sb = stp.tile([128, T], dt.bfloat16)
            dstB_sb = stp.tile([128, T], dt.bfloat16)
            mflg_sb = stp.tile([128, T], dt.float8e4)
            nc.sync.dma_start(out=dstA_sb[:], in_=dstA[:])
            nc.sync.dma_start(out=dstB_sb[:], in_=dstB[:])
            nc.sync.dma_start(out=mflg_sb[:], in_=mflg[:])

            # big activations
            zT = [bigp.tile([128, NLOC], dt.bfloat16, tag=f"zT{e}", name=f"zT{e}") for e in range(2)]
            z2T = [bigp.tile([128, NLOC], dt.bfloat16, tag=f"z2T{e}", name=f"z2T{e}")
                   for e in range(2)]
            hT = [bigp.tile([128, NLOC], dt.bfloat16, tag=f"hT{e}", name=f"hTe{e}") for e in range(2)]
            fusedA = {l: {"ysq": {}, "ysums": {}, "arr": {}} for l in range(2)}
            for l in range(2):
                for e in range(2):
                    fusedA[l]["ysq"][e] = statp.tile(
                        [128, 2 * (NBLK // BLK_GRP + 1)], dt.float32,
                        tag=f"fysq{l}{e}", name=f"fysq{l}{e}")
                    fusedA[l]["ysums"][e] = statp.tile(
                        [128, 2 * (NBLK // BLK_GRP + 1)], dt.float32,
                        tag=f"fysums{l}{e}", name=f"fysums{l}{e}")
            pools = statp.tile([128, 4], dt.float32)  # cols: e0l0,e0l1,e1l0,e1l1
            wcos_acc = statp.tile([128, 1], dt.float32)
            nc.vector.memset(pools[:], 0.0)
            nc.vector.memset(wcos_acc[:], 0.0)

            def all_reduce_stats(stats_tile, ncols):
                """DMA stats [128, ncols] -> AR -> return SBUF tile with result."""
                ar_i[0] += 1
                ari = dramp.tile([128, 8], dt.float32, tag="arin")
                aro = dramp.tile([128, 8], dt.float32, tag="arout", addr_space="Shared")
                nc.sync.dma_start(out=ari[:, :ncols], in_=stats_tile[:, :ncols])
                if ncols < 8:
                    zpad = smp.tile([128, 8], dt.float32, tag="zpad")
                    nc.vector.memset(zpad[:], 0.0)
                    nc.sync.dma_start(out=ari[:, ncols:], in_=zpad[:, ncols:])
                nc.gpsimd.collective_compute(
                    "AllReduce", mybir.AluOpType.add, replica_groups=rg,
                    ins=[ari[:].opt()], outs=[aro[:].opt()])
                res = smp.tile([128, 8], dt.float32, tag="arres")
                nc.sync.dma_start(out=res[:, :ncols], in_=aro[:, :ncols])
                return res

            # ---------------- edge phase ----------------
            def edge_phase(layer):
                """Produces zT[0], zT[1] = own + agg for both encoders."""
                ctx = nc.named_scope(f"edge{layer}"); ctx.__enter__()
                masked = (layer == 0)
                src = src_sb[layer]
                with tc.tile_pool(name=f"psum_e{layer}", bufs=2, space="PSUM") as pp, \
                     tc.tile_pool(name=f"psum_a{layer}", bufs=1, space="PSUM") as ppA, \
                     tc.tile_pool(name=f"own{layer}", bufs=3) as ownp:
                    ngrpA = (NBLK + BLK_GRP - 1) // BLK_GRP

                    def passA_group(gidx):
                        gsz = min(BLK_GRP, NBLK - gidx * BLK_GRP)
                        ncols = gsz * 128
                        c0 = gidx * BLK_GRP * 128
                        for e in (1, 0):
                            p = wt[e, layer]
                            for h in range(2):
                                psy = ppA.tile([128, 512], dt.float32, space="PSUM",
                                               tag=f"psyA{h}", name=f"psyA{h}")
                                nc.tensor.matmul(out=psy[:, :ncols],
                                                 lhsT=p["w1"][:, h * 128:(h + 1) * 128],
                                                 rhs=zT[e][:, c0:c0 + ncols],
                                                 start=True, stop=True)
                                deadA = smp.tile([128, 512], dt.bfloat16,
                                                 tag="deadA", name="deadA")
                                nc.scalar.activation(
                                    out=deadA[:, :ncols], in_=psy[:, :ncols],
                                    func=mybir.ActivationFunctionType.Square,
                                    accum_out=fusedA[layer]["ysq"][e][:, 2 * gidx + h:2 * gidx + h + 1])
                                nc.vector.tensor_reduce(
                                    out=fusedA[layer]["ysums"][e][:, 2 * gidx + h:2 * gidx + h + 1],
                                    in_=psy[:, :ncols],
                                    axis=mybir.AxisListType.X, op=mybir.AluOpType.add)

                    psA = psB = psC = None
                    for t in range(T):
                        w = int(win_of_tile[t])
                        if masked:
                            gbuf = gp.tile([128, D], dt.float8e4, tag="gA", name="gA")
                            nc.gpsimd.indirect_dma_start(
                                out=gbuf[:], out_offset=None, in_=feat_tab[:],
                                in_offset=bass.IndirectOffsetOnAxis(
                                    ap=src[:, t:t + 1], axis=0))
                        else:
                            gbuf = gp.tile([128, 2 * D], dt.float8e4, tag="gA", name="gA")
                            nc.gpsimd.indirect_dma_start(
                                out=gbuf[:], out_offset=None, in_=htab[:],
                                in_offset=bass.IndirectOffsetOnAxis(
                                    ap=src[:, t:t + 1], axis=0))
                        if t == first_of_win[w]:
                            psA = pp.tile([128, WW], dt.float32, space="PSUM", tag="psA")
                            psB = pp.tile([128, WW], dt.float32, space="PSUM", tag="psB")
                            if masked:
                                psC = pp.tile([1, WW], dt.float32, space="PSUM", tag="psC")
                        first = (t == first_of_win[w])
                        last = (t == last_of_win[w])
                        g_sl = gbuf[:, 0:D]
                        # S for unmasked (encoder 2 target / both in layer 1)
                        Sf = sti.tile([128, WW], dt.float8e4,
                                      tag="Sf", name="Sf")
                        nc.vector.tensor_tensor(
                            out=Sf[:], in0=dstA_sb[:, t:t + 1].to_broadcast([128, WW]),
                            in1=iota_t[:], op=mybir.AluOpType.is_equal)
                        if masked:
                            Sx = sti.tile([128, WW], dt.float8e4, tag="Sx")
                            nc.vector.tensor_tensor(
                                out=Sx[:], in0=dstB_sb[:, t:t + 1].to_broadcast([128, WW]),
                                in1=iota_t[:], op=mybir.AluOpType.is_equal)
                            # encoder1 agg (masked inputs): S_x matmul
                            nc.tensor.matmul(out=psA[:], lhsT=g_sl, rhs=Sx[:],
                                             start=first, stop=False)
                            # count of masked in-edges per window node
                            nc.tensor.matmul(out=psC[:], lhsT=mflg_sb[:, t:t + 1], rhs=Sf[:],
                                             start=first, stop=last)
                            # encoder2 agg (unmasked)
                            nc.tensor.matmul(out=psB[:], lhsT=g_sl, rhs=Sf[:],
                                             start=first, stop=last)
                        else:
                            nc.tensor.matmul(out=psA[:], lhsT=g_sl, rhs=Sf[:],
                                             start=first, stop=last)
                            nc.tensor.matmul(out=psB[:], lhsT=gbuf[:, D:2 * D],
                                             rhs=Sf[:], start=first, stop=last)
                        if last:
                            cols = slice(w * WW, (w + 1) * WW)
                            if masked:
                                cnt_sb = smp.tile([1, WW], dt.bfloat16, tag="cnt")
                                nc.vector.tensor_copy(out=cnt_sb[:], in_=psC[:])
                                nc.tensor.matmul(out=psA[:], lhsT=ttok_t[:], rhs=cnt_sb[:],
                                                 start=False, stop=True)
                                own1 = ownp.tile([128, WW], dt.bfloat16, tag="own1")
                                own2 = ownp.tile([128, WW], dt.bfloat16, tag="own2")
                                nc.sync.dma_start(out=own1[:], in_=xT_own[:, cols])
                                nc.sync.dma_start(out=own2[:], in_=fT_own[:, cols])
                            else:
                                own1 = hT[0][:, cols]
                                own2 = hT[1][:, cols]
                            a1 = smp.tile([128, WW], dt.bfloat16, tag="a1")
                            a2 = smp.tile([128, WW], dt.bfloat16, tag="a2")
                            nc.scalar.activation(out=a1[:], in_=psA[:],
                                                 func=mybir.ActivationFunctionType.Copy)
                            nc.scalar.activation(out=a2[:], in_=psB[:],
                                                 func=mybir.ActivationFunctionType.Copy)
                            o1 = own1[:] if masked else own1
                            o2 = own2[:] if masked else own2
                            nc.vector.tensor_add(out=zT[0][:, cols], in0=a1[:], in1=o1)
                            nc.vector.tensor_add(out=zT[1][:, cols], in0=a2[:], in1=o2)
                            if w % 2 == 1:
                                passA_group(w // 2)
                            elif w == WIN - 1:
                                passA_group(w // 2)
                    # final stat reduces + AR issue for both encoders
                    for e in (1, 0):
                        st = statp.tile([128, 4], dt.float32, tag="bnstat", name="bnstat")
                        for h in range(2):
                            nc.vector.tensor_reduce(
                                out=st[:, h:h + 1],
                                in_=fusedA[layer]["ysums"][e][:, h:2 * ngrpA:2],
                                axis=mybir.AxisListType.X, op=mybir.AluOpType.add)
                            nc.vector.tensor_reduce(
                                out=st[:, 2 + h:3 + h],
                                in_=fusedA[layer]["ysq"][e][:, h:2 * ngrpA:2],
                                axis=mybir.AxisListType.X, op=mybir.AluOpType.add)
                        fusedA[layer]["arr"][e] = all_reduce_stats(st, 4)

                ctx.__exit__(None, None, None)

            # ---------------- node phase ----------------
            def node_phase(e, layer, write_table, pp, nsp):
                p = wt[e, layer]
                z = zT[e]
                with contextlib.nullcontext(pp) as pp, \
                     contextlib.nullcontext(nsp) as nsp:
                    ngrp = (NBLK + BLK_GRP - 1) // BLK_GRP
                    arr = fusedA[layer]["arr"][e]
                    yield
                    # affine: s = g/sqrt(var+eps), t = b - mean*s  (per half)
                    s1 = smp.tile([128, 2], dt.float32, tag="s1")
                    t1 = smp.tile([128, 2], dt.float32, tag="t1")
                    mean = smp.tile([128, 2], dt.float32, tag="mean")
                    var = smp.tile([128, 2], dt.float32, tag="var")
                    nc.vector.tensor_scalar_mul(mean[:], arr[:, 0:2], 1.0 / N)
                    nc.vector.tensor_scalar_mul(var[:], arr[:, 2:4], 1.0 / N)
                    msq = smp.tile([128, 2], dt.float32, tag="msq")
                    nc.vector.tensor_mul(msq[:], mean[:], mean[:])
                    nc.vector.tensor_sub(var[:], var[:], msq[:])
                    nc.vector.tensor_scalar_add(var[:], var[:], BN_EPS)
                    nc.vector.reciprocal(var[:], var[:])
                    nc.scalar.activation(out=var[:], in_=var[:],
                                         func=mybir.ActivationFunctionType.Sqrt)
                    nc.vector.tensor_mul(s1[:], p["g1"][:], var[:])
                    nc.vector.tensor_mul(t1[:], mean[:], s1[:])
                    nc.vector.tensor_sub(t1[:], p["b1"][:], t1[:])

                    # ---- pass B: recompute y, BN1+relu, W2 -> z2, stats2 ----
                    z2sum = nsp.tile([128, NBLK // BLK_GRP + 1], dt.float32, tag="z2sum")
                    z2sq = nsp.tile([128, NBLK // BLK_GRP + 1], dt.float32, tag="z2sq")
                    for g in range(ngrp):
                        gsz = min(BLK_GRP, NBLK - g * BLK_GRP)
                        ncols = gsz * 128
                        c0 = g * BLK_GRP * 128
                        zsl = z[:, c0:c0 + ncols]
                        psz = pp.tile([128, 512], dt.float32, space="PSUM", tag="psz")
                        for h in range(2):
                            psy = pp.tile([128, 512], dt.float32, space="PSUM", tag=f"psy{h}")
                            nc.tensor.matmul(out=psy[:, :ncols],
                                             lhsT=p["w1"][:, h * 128:(h + 1) * 128],
                                             rhs=zsl, start=True, stop=True)
                            hm = nsp.tile([128, 512], dt.bfloat16, tag=f"hm{h}")
                            nc.scalar.activation(out=hm[:, :ncols], in_=psy[:, :ncols],
                                                 func=mybir.ActivationFunctionType.Relu,
                                                 bias=t1[:, h:h + 1], scale=s1[:, h:h + 1])
                            nc.tensor.matmul(out=psz[:, :ncols],
                                             lhsT=p["w2a" if h == 0 else "w2b"][:],
                                             rhs=hm[:, :ncols], start=(h == 0), stop=(h == 1))
                        nc.vector.tensor_copy(out=z2T[e][:, c0:c0 + ncols], in_=psz[:, :ncols])
                        # exclude padded phantom nodes (cols >= NSHARD) from stats
                        vcols = min(ncols, NSHARD - c0)
                        dead = nsp.tile([128, 512], dt.bfloat16, tag="dead")
                        nc.scalar.activation(out=dead[:, :vcols], in_=psz[:, :vcols],
                                             func=mybir.ActivationFunctionType.Square,
                                             accum_out=z2sq[:, g:g + 1])
                        nc.vector.tensor_reduce(
                            out=z2sum[:, g:g + 1], in_=psz[:, :vcols],
                            axis=mybir.AxisListType.X, op=mybir.AluOpType.add)
                    if NLOC > NSHARD:
                        nc.vector.memset(z2T[e][:, NSHARD:NLOC], 0.0)
                    st2 = statp.tile([128, 4], dt.float32, tag="bnstat2")
                    nc.vector.tensor_reduce(out=st2[:, 0:1], in_=z2sum[:, :ngrp],
                                            axis=mybir.AxisListType.X, op=mybir.AluOpType.add)
                    nc.vector.tensor_reduce(out=st2[:, 1:2], in_=z2sq[:, :ngrp],
                                            axis=mybir.AxisListType.X, op=mybir.AluOpType.add)
                    arr2 = all_reduce_stats(st2, 2)
                    yield
                    s2 = smp.tile([128, 1], dt.float32, tag="s2")
                    t2 = smp.tile([128, 1], dt.float32, tag="t2")
                    mean2 = smp.tile([128, 1], dt.float32, tag="mean2")
                    var2 = smp.tile([128, 1], dt.float32, tag="var2")
                    nc.vector.tensor_scalar_mul(mean2[:], arr2[:, 0:1], 1.0 / N)
                    nc.vector.tensor_scalar_mul(var2[:], arr2[:, 1:2], 1.0 / N)
                    msq2 = smp.tile([128, 1], dt.float32, tag="msq2")
                    nc.vector.tensor_mul(msq2[:], mean2[:], mean2[:])
                    nc.vector.tensor_sub(var2[:], var2[:], msq2[:])
                    nc.vector.tensor_scalar_add(var2[:], var2[:], BN_EPS)
                    nc.vector.reciprocal(var2[:], var2[:])
                    nc.scalar.activation(out=var2[:], in_=var2[:],
                                         func=mybir.ActivationFunctionType.Sqrt)
                    nc.vector.tensor_mul(s2[:], p["g2"][:], var2[:])
                    nc.vector.tensor_mul(t2[:], mean2[:], s2[:])
                    nc.vector.tensor_sub(t2[:], p["b2"][:], t2[:])

                    # ---- pass C: h = relu(BN2(z2)), pools, optional table ----
                    poolstage = nsp.tile([128, NBLK // BLK_GRP + 1], dt.float32, tag="pst")
                    for g in range(ngrp):
                        gsz = min(BLK_GRP, NBLK - g * BLK_GRP)
                        ncols = gsz * 128
                        c0 = g * BLK_GRP * 128
                        nc.scalar.activation(out=hT[e][:, c0:c0 + ncols],
                                             in_=z2T[e][:, c0:c0 + ncols],
                                             func=mybir.ActivationFunctionType.Relu,
                                             bias=t2[:, 0:1], scale=s2[:, 0:1],
                                             accum_out=poolstage[:, g:g + 1])
                    # pools: but last group includes padded nodes (cols 12500..12544
                    # of window 97); those z2 are 0 -> h = relu(t2) possibly != 0.
                    # subtract the pad contribution: npad * relu(t2)
                    npad = NLOC - NSHARD
                    relut2 = smp.tile([128, 1], dt.float32, tag="relut2")
                    nc.vector.tensor_scalar_max(relut2[:], t2[:], 0.0)
                    nc.vector.tensor_scalar_mul(relut2[:], relut2[:], -float(npad))
                    pcol = pools[:, 2 * e + layer:2 * e + layer + 1]
                    nc.vector.tensor_reduce(out=pcol, in_=poolstage[:, :ngrp],
                                            axis=mybir.AxisListType.X, op=mybir.AluOpType.add)
                    nc.vector.tensor_add(out=pcol, in0=pcol, in1=relut2[:])
                    # also zero out the pad columns of hT so layer-1 z is clean
                    if NLOC > NSHARD:
                        nc.vector.memset(hT[e][:, NSHARD:NLOC], 0.0)

                    if write_table:
                        with tc.tile_pool(name=f"tr{e}", bufs=3) as trp, \
                             tc.tile_pool(name=f"psum_tr{e}", bufs=2, space="PSUM") as ptp:
                            for g in range(ngrp):
                                gsz = min(BLK_GRP, NBLK - g * BLK_GRP)
                                stg = trp.tile([128, 512], dt.float8e4, tag="stg")
                                for jj in range(gsz):
                                    blk = g * BLK_GRP + jj
                                    pst = ptp.tile([128, 128], dt.bfloat16, space="PSUM", tag="pt")
                                    nc.tensor.transpose(
                                        out=pst[:], in_=hT[e][:, blk * 128:(blk + 1) * 128],
                                        identity=ident_t[:])
                                    nc.vector.tensor_copy(out=stg[:, jj * 128:(jj + 1) * 128],
                                                          in_=pst[:])
                                nc.sync.dma_start(
                                    out=htab_loc[g * 512:g * 512 + gsz * 128,
                                                  e * D:(e + 1) * D].rearrange(
                                        "(c p) f -> p c f", p=128),
                                    in_=stg[:, :gsz * 128].rearrange("p (c f) -> p c f", f=128))

            def run_node_layer(layer, write_table):
                with tc.tile_pool(name=f"psum_n{layer}", bufs=2, space="PSUM") as pp, \
                     tc.tile_pool(name=f"nstage{layer}", bufs=3) as nsp:
                    g1 = node_phase(1, layer, write_table, pp, nsp)
                    g0 = node_phase(0, layer, write_table, pp, nsp)
                    next(g1); next(g0)
                    next(g1); next(g0)
                    for _ in g1:
                        pass
                    for _ in g0:
                        pass

            # ---------------- run the pipeline ----------------
            edge_phase(0)
            if DEBUG:
                nc.sync.dma_start(out=dbg["zt0"][:], in_=zT[0][:])
                nc.sync.dma_start(out=dbg["zt1"][:], in_=zT[1][:])
            run_node_layer(0, write_table=True)
            nc.gpsimd.collective_compute(
                "AllGather", mybir.AluOpType.bypass, replica_groups=rg,
                ins=[htab_loc[:].opt()], outs=[htab[:].opt()])
            if DEBUG:
                nc.sync.dma_start(out=dbg["h0"][:], in_=hT[0][:])
                nc.sync.dma_start(out=dbg["h1"][:], in_=hT[1][:])
            edge_phase(1)
            if DEBUG:
                nc.sync.dma_start(out=dbg["zl0"][:], in_=zT[0][:])
                nc.sync.dma_start(out=dbg["zl1"][:], in_=zT[1][:])
            run_node_layer(1, write_table=False)
            if DEBUG:
                nc.sync.dma_start(out=dbg["hf0"][:], in_=hT[0][:])
                nc.sync.dma_start(out=dbg["hf1"][:], in_=hT[1][:])

            # ---------------- loss ----------------
            with tc.tile_pool(name="psum_l", bufs=2, space="PSUM") as plp, \
                 tc.tile_pool(name="lstage", bufs=3) as lsp:
                A = lsp.tile([128, NBLK], dt.float32, tag="A")
                B = lsp.tile([128, NBLK], dt.float32, tag="B")
                C = lsp.tile([128, NBLK], dt.float32, tag="C")
                ngrp = (NBLK + BLK_GRP - 1) // BLK_GRP
                for g in range(ngrp):
                    gsz = min(BLK_GRP, NBLK - g * BLK_GRP)
                    ncols = gsz * 128
                    c0 = g * BLK_GRP * 128
                    u = lsp.tile([128, 512], dt.bfloat16, tag="u")
                    q1 = lsp.tile([128, 512], dt.bfloat16, tag="q1")
                    q2 = lsp.tile([128, 512], dt.bfloat16, tag="q2")
                    nc.vector.tensor_mul(u[:, :ncols], hT[0][:, c0:c0 + ncols],
                                         hT[1][:, c0:c0 + ncols])
                    nc.vector.tensor_mul(q1[:, :ncols], hT[0][:, c0:c0 + ncols],
                                         hT[0][:, c0:c0 + ncols])
                    nc.vector.tensor_mul(q2[:, :ncols], hT[1][:, c0:c0 + ncols],
                                         hT[1][:, c0:c0 + ncols])
                    pa = plp.tile([128, BLK_GRP], dt.float32, space="PSUM", tag="pa")
                    pb = plp.tile([128, BLK_GRP], dt.float32, space="PSUM", tag="pb")
                    pc2 = plp.tile([128, BLK_GRP], dt.float32, space="PSUM", tag="pc2")
                    for jj in range(gsz):
                        sl = slice(jj * 128, (jj + 1) * 128)
                        nc.tensor.matmul(out=pa[:, jj:jj + 1], lhsT=u[:, sl], rhs=ones_t[:],
                                         start=True, stop=True)
                        nc.tensor.matmul(out=pb[:, jj:jj + 1], lhsT=q1[:, sl], rhs=ones_t[:],
                                         start=True, stop=True)
                        nc.tensor.matmul(out=pc2[:, jj:jj + 1], lhsT=q2[:, sl], rhs=ones_t[:],
                                         start=True, stop=True)
                    gs = slice(g * BLK_GRP, g * BLK_GRP + gsz)
                    nc.vector.tensor_copy(out=A[:, gs], in_=pa[:, :gsz])
                    nc.vector.tensor_copy(out=B[:, gs], in_=pb[:, :gsz])
                    nc.vector.tensor_copy(out=C[:, gs], in_=pc2[:, :gsz])
                # wcos = w * a / sqrt(b*c)
                BC = lsp.tile([128, NBLK], dt.float32, tag="BC")
                nc.vector.tensor_mul(BC[:], B[:], C[:])
                nc.vector.tensor_scalar_max(BC[:], BC[:], 1e-24)
                nc.vector.reciprocal(BC[:], BC[:])
                nc.scalar.activation(out=BC[:], in_=BC[:],
                                     func=mybir.ActivationFunctionType.Sqrt)
                nc.vector.tensor_mul(BC[:], BC[:], A[:])
                nc.vector.tensor_mul(BC[:], BC[:], wnode_t[:])
                nc.vector.tensor_reduce(out=wcos_acc[:], in_=BC[:],
                                        axis=mybir.AxisListType.X, op=mybir.AluOpType.add)

                # final AR: [pools(4) | wcos(1)] -> [128, 5]... use 4+1 in two tiles
                if DEBUG:
                    nc.sync.dma_start(out=dbg["pools"][:, 0:4], in_=pools[:])
                    nc.sync.dma_start(out=dbg["pools"][:, 4:5], in_=wcos_acc[:])
                fin = statp.tile([128, 5], dt.float32, tag="fin")
                nc.vector.tensor_copy(out=fin[:, 0:4], in_=pools[:])
                nc.vector.tensor_copy(out=fin[:, 4:5], in_=wcos_acc[:])
                arr_p = all_reduce_stats(fin, 5)
                arr_w = arr_p[:, 4:5]
                # global scalars via ones-matmul: stack products
                stack = lsp.tile([128, 4], dt.float32, tag="stack")
                # dot = sum_f p0l0*p1l0 + p0l1*p1l1 ; n1 = p0^2 ; n2 = p1^2
                t_a = lsp.tile([128, 2], dt.float32, tag="t_a")
                t_b = lsp.tile([128, 2], dt.float32, tag="t_b")
                t_c = lsp.tile([128, 2], dt.float32, tag="t_c")
                nc.vector.tensor_mul(t_a[:], arr_p[:, 0:2], arr_p[:, 2:4])
                nc.vector.tensor_mul(t_b[:], arr_p[:, 0:2], arr_p[:, 0:2])
                nc.vector.tensor_mul(t_c[:], arr_p[:, 2:4], arr_p[:, 2:4])
                nc.vector.tensor_reduce(out=stack[:, 0:1], in_=t_a[:],
                                        axis=mybir.AxisListType.X, op=mybir.AluOpType.add)
                nc.vector.tensor_reduce(out=stack[:, 1:2], in_=t_b[:],
                                        axis=mybir.AxisListType.X, op=mybir.AluOpType.add)
                nc.vector.tensor_reduce(out=stack[:, 2:3], in_=t_c[:],
                                        axis=mybir.AxisListType.X, op=mybir.AluOpType.add)
                nc.vector.tensor_copy(out=stack[:, 3:4], in_=arr_w)
                stack_bf = lsp.tile([128, 4], dt.float32, tag="stackbf")
                nc.vector.tensor_copy(out=stack_bf[:], in_=stack[:])
                pfin = plp.tile([1, 4], dt.float32, space="PSUM", tag="pfin")
                ones_f = lsp.tile([128, 1], dt.float32, tag="onesf")
                nc.vector.memset(ones_f[:], 1.0)
                for jj in range(4):
                    nc.tensor.matmul(out=pfin[:, jj:jj + 1], lhsT=stack_bf[:, jj:jj + 1],
                                     rhs=ones_f[:], start=True, stop=True)
                # loss = 0.5*(1 - wcos/Mn) + 0.5*(1 - dot/sqrt(n1*n2))
                sc = lsp.tile([1, 4], dt.float32, tag="sc")
                nc.vector.tensor_copy(out=sc[:], in_=pfin[:])
                g2v = lsp.tile([1, 1], dt.float32, tag="g2v")
                nc.vector.tensor_mul(g2v[:], sc[:, 1:2], sc[:, 2:3])
                nc.vector.tensor_scalar_max(g2v[:], g2v[:], 1e-24)
                nc.vector.reciprocal(g2v[:], g2v[:])
                nc.scalar.activation(out=g2v[:], in_=g2v[:],
                                     func=mybir.ActivationFunctionType.Sqrt)
                nc.vector.tensor_mul(g2v[:], g2v[:], sc[:, 0:1])  # cos_g
                res = lsp.tile([1, 1], dt.float32, tag="res")
                nc.vector.tensor_scalar_mul(res[:], sc[:, 3:4], -0.5 / float(Mn))
                nc.vector.tensor_scalar_add(res[:], res[:], 1.0)
                half = lsp.tile([1, 1], dt.float32, tag="half")
                nc.vector.tensor_scalar_mul(half[:], g2v[:], 0.5)
                nc.vector.tensor_sub(res[:], res[:], half[:])
                nc.sync.dma_start(out=out_loss[:], in_=res[:])

    nc.compile()
    return nc


# ---------------- public entry ----------------
_CACHE = {}


def prepare(feat, mask_token, oW1, oW2, og1, ob1, og2, ob2,
            tW1, tW2, tg1, tb1, tg2, tb2, src, dst, mask_nodes):
    feat = np.asarray(feat, np.float32)
    mask_token = np.asarray(mask_token, np.float32)
    src = np.asarray(src, np.int32)
    dst = np.asarray(dst, np.int32)
    mask_nodes = np.asarray(mask_nodes, np.int32)

    pp = preprocess(src, dst, mask_nodes)
    T = pp["T"]

    key = (T, len(mask_nodes))
    if key not in _CACHE:
        _CACHE[key] = build(T, pp["win_of_tile"], len(mask_nodes))
    nc = _CACHE[key]

    # host tensors
    x = feat.copy()
    x[mask_nodes] = np.broadcast_to(mask_token, (len(mask_nodes), D))
    feat_bf = feat.astype(ml_dtypes.float8_e4m3)
    iota_c = np.broadcast_to(np.arange(WW, dtype=np.float32), (128, WW)).astype(bf16)
    ones_c = np.ones((128, 1), bf16)
    ident_c = np.eye(128, dtype=np.float32).astype(bf16)
    ttok_c = mask_token.reshape(1, D).astype(bf16)
    is_m = np.bincount(mask_nodes, minlength=N).astype(np.float32)

    enc_params = [
        (oW1, oW2, og1, ob1, og2, ob2),   # e=0 online (masked input)
        (tW1, tW2, tg1, tb1, tg2, tb2),   # e=1 target
    ]

    common = {"feat_tab": feat_bf, "iota_c": iota_c, "ones_c": ones_c,
              "ident_c": ident_c, "ttok_c": ttok_c}
    for e in range(2):
        w1, w2, g1, b1, g2, b2 = enc_params[e]
        for l in range(2):
            common[f"w1_{e}{l}"] = np.asarray(w1[l], np.float32).astype(bf16)
            common[f"w2_{e}{l}"] = np.asarray(w2[l], np.float32).astype(bf16)
            common[f"g1_{e}{l}"] = np.asarray(g1[l], np.float32).reshape(2, 128).T.copy()
            common[f"b1_{e}{l}"] = np.asarray(b1[l], np.float32).reshape(2, 128).T.copy()
            common[f"g2_{e}{l}"] = np.asarray(g2[l], np.float32).reshape(128, 1).copy()
            common[f"b2_{e}{l}"] = np.asarray(b2[l], np.float32).reshape(128, 1).copy()

    in_maps = []
    for c in range(NCORES):
        rows = slice(c * NSHARD, (c + 1) * NSHARD)
        xT = np.zeros((128, NLOC), np.float32)
        fT = np.zeros((128, NLOC), np.float32)
        xT[:, :NSHARD] = x[rows].T
        fT[:, :NSHARD] = feat[rows].T
        wn = np.zeros(NLOC, np.float32)
        wn[:NSHARD] = is_m[rows]
        m = dict(common)
        m.update({
            "xT_own": xT.astype(bf16), "fT_own": fT.astype(bf16),
            "src0": pp["src0"][c], "src1": pp["src1"][c],
            "dstA": pp["dstA"][c], "dstB": pp["dstB"][c], "mflg": pp["mflg"][c],
            "w_node": wn.reshape(NBLK, 128).T.copy(),
        })
        in_maps.append(m)

    return nc, in_maps


def kernel(**inputs):
    nc, in_maps = prepare(**inputs)
    last_err = None
    for attempt in range(3):
        try:
            res = run_bass_kernel_spmd(nc, in_maps, core_ids=list(range(NCORES)))
            loss = res.results[0]["loss"].reshape(())
            return np.float32(loss)
        except Exception as e:  # transient NRT device errors happen; retry
            last_err = e
    raise last_err



# revision 12
# speedup vs baseline: 1.1309x; 1.1309x over previous
"""GraphMAE-style 2-layer GIN loss kernel for one TRN2 chip (8 NeuronCores).

Self-contained: builds + compiles a Bass/Tile SPMD kernel, shards the graph on
the host (dst-partitioned nodes + incident edges), runs on 8 cores via
run_bass_kernel_spmd, returns the scalar loss.

Design (v2, optimized):
  - nodes sharded: core c owns rows [12500c, 12500(c+1)), padded to 12544
  - gather tables hold BOTH encoders' features per row (256B fp8):
    layer 0: [x | feat], layer 1: [h0 | h1]; one batched indirect DMA
    gathers GRP*128 edges per instruction (SWDGE cost is ~1us fixed +
    0.34ns/row, so batching is ~6x cheaper than per-tile gathers)
  - segment_sum: per dst-window of 256 nodes, PSUM accumulates
    gathered_rows.T @ onehot(dst); the node's own contribution is seeded
    into PSUM with an identity-weight matmul (z = own + agg)
  - one-hot built by DVE tensor_scalar is_equal (bf16, 4x perf mode)
  - BN1 stats: sum(y) = W1.T @ sum(z) (tiny matmul after AllReduce);
    sum(y^2) accumulated by ScalarE Square activations during edge phase
  - h tables written transposed (fp8) to a shared DRAM table via a
    4-chunk AllGather overlapped with the transpose writes
  - feat-major activations [128 feat, nodes]; BN affine + relu on ScalarE
"""
import sys, os, contextlib
sys.path.insert(0, '/opt/trn_rl_repo')
import numpy as np
import ml_dtypes

from concourse import bass, bacc, tile, mybir
from concourse.bass_utils import run_bass_kernel_spmd

dt = mybir.dt
bf16 = ml_dtypes.bfloat16
f8 = ml_dtypes.float8_e4m3

# ---------------- problem constants ----------------
N = 100000
E = 800000
D = 128
H = 256
L = 2
NCORES = 8
NSHARD = N // NCORES          # 12500
NLOC = 12544                  # padded to 98*128
WW = 256                      # dst-window width (nodes)
WIN = NLOC // WW              # 49 windows per core
NBLK = NLOC // 128            # 98 node blocks per core
BN_EPS = 1e-5
BLK_GRP = 4                   # node blocks per matmul group (free dim 512)
NGRP = (NBLK + BLK_GRP - 1) // BLK_GRP   # 25


# ---------------- host-side graph preprocessing ----------------
def preprocess(src, dst):
    """Build per-core edge streams + shared schedule, one per layer.

    Stream order: (window-pair, bin, window, src).  Each gather instruction
    covers the tiles of one (pair, bin) run; int16 indices are bin-local.
    """
    src = np.asarray(src).astype(np.int64)
    dst = np.asarray(dst).astype(np.int64)
    core_of = dst // NSHARD
    dst_loc = dst % NSHARD
    win = dst_loc // WW
    dloc = dst_loc % WW
    NPAIR = (WIN + 1) // 2

    out = []
    for layer in range(2):
        if layer == 0:
            row = src
            nrow = N
        else:
            row = (src // NSHARD) * NLOC + src % NSHARD
            nrow = NLOC * NCORES
        nbin = (nrow + 32767) // 32768
        bin_ = row // 32768
        loc16 = (row - bin_ * 32768).astype(np.int64)

        # per-core counts per (win, bin)
        counts = np.zeros((NCORES, WIN, nbin), np.int64)
        sels = []
        for c in range(NCORES):
            sel = np.nonzero(core_of == c)[0]
            order = np.lexsort((src[sel], bin_[sel], win[sel]))
            sel = sel[order]
            sels.append(sel)
            np.add.at(counts[c], (win[sel], bin_[sel]), 1)
        ntile = (counts.max(axis=0) + 127) // 128      # [WIN, nbin]

        # tile stream ordered (pair, bin, win); instruction per (pair, bin)
        win_of_tile = []
        instrs = []            # (t0, nt, bin)
        tile_start = np.zeros((WIN, nbin), np.int64)
        for pr in range(NPAIR):
            ws = [2 * pr] + ([2 * pr + 1] if 2 * pr + 1 < WIN else [])
            for b in range(nbin):
                nt = int(sum(ntile[w, b] for w in ws))
                if nt == 0:
                    continue
                t0 = len(win_of_tile)
                # dma_gather is limited to 1024 indices (8 tiles) per instr
                for o in range(0, nt, 8):
                    instrs.append((t0 + o, min(8, nt - o), b))
                for w in ws:
                    tile_start[w, b] = len(win_of_tile)
                    win_of_tile += [w] * int(ntile[w, b])
        T = len(win_of_tile)
        win_of_tile = np.array(win_of_tile, np.int64)

        idx16 = np.zeros((NCORES, 128, T * 8), np.int16)
        dstA = np.full((NCORES, T * 128), -1.0, np.float32)
        for c in range(NCORES):
            sel = sels[c]
            # rank within (win, bin) group
            ofs = np.concatenate([[0], np.cumsum(counts[c].ravel())])
            key = win[sel] * nbin + bin_[sel]
            pos = tile_start[win[sel], bin_[sel]] * 128 + \
                (np.arange(len(sel)) - ofs[key])
            # idx stream: edge at stream pos i -> [16*k + i%16, i//16]
            rows = (pos % 16).astype(np.int64)
            cols = (pos // 16).astype(np.int64)
            for k in range(8):
                idx16[c, 16 * k + rows, cols] = loc16[sel]
            dstA[c, pos] = dloc[sel]
        dstA = np.ascontiguousarray(
            dstA.reshape(NCORES, T, 128).transpose(0, 2, 1)).astype(np.float32)
        out.append(dict(T=T, win_of_tile=win_of_tile, instrs=instrs,
                        nbin=nbin, idx16=idx16, dstA=dstA))
    return out


# ---------------- kernel builder ----------------
def build(sched, Mn):
    nc = bacc.Bacc("TRN2", target_bir_lowering=False, debug=False,
                   num_devices=NCORES)
    rg = [list(range(NCORES))]

    def inp(name, shape, d):
        return nc.dram_tensor(name, shape, d, kind="ExternalInput")

    xf_tab = inp("xf_tab", [N, 2 * D], dt.float8e4)
    xT_own = inp("xT_own", [128, NLOC], dt.bfloat16)
    fT_own = inp("fT_own", [128, NLOC], dt.bfloat16)
    idx_in = [inp(f"idx{l}", [128, sched[l]["T"] * 8], dt.int16)
              for l in range(2)]
    dst_in = [inp(f"dst{l}", [128, sched[l]["T"]], dt.float32)
              for l in range(2)]
    iota_c = inp("iota_c", [128, WW], dt.bfloat16)
    ones_c = inp("ones_c", [128, 1], dt.bfloat16)
    ident_c = inp("ident_c", [128, 128], dt.bfloat16)
    w_node = inp("w_node", [128, NBLK], dt.float32)
    W1 = {}; W2 = {}; G1 = {}; B1 = {}; G2 = {}; B2 = {}
    for e in range(2):
        for l in range(2):
            W1[e, l] = inp(f"w1_{e}{l}", [128, H], dt.bfloat16)
            W2[e, l] = inp(f"w2_{e}{l}", [H, 128], dt.bfloat16)
            G1[e, l] = inp(f"g1_{e}{l}", [128, 2], dt.float32)
            B1[e, l] = inp(f"b1_{e}{l}", [128, 2], dt.float32)
            G2[e, l] = inp(f"g2_{e}{l}", [128, 1], dt.float32)
            B2[e, l] = inp(f"b2_{e}{l}", [128, 1], dt.float32)
    out_loss = nc.dram_tensor("loss", [1, 1], dt.float32, kind="ExternalOutput")
    DEBUG = bool(int(os.environ.get("GIN_DEBUG", "0")))
    dbg = {}
    if DEBUG:
        for nm in ["zt0", "zt1", "h0", "h1", "zl0", "zl1", "hf0", "hf1"]:
            dbg[nm] = nc.dram_tensor(f"dbg_{nm}", [128, NLOC], dt.bfloat16,
                                     kind="ExternalOutput")
        dbg["st"] = nc.dram_tensor("dbg_st", [128, 8], dt.float32,
                                   kind="ExternalOutput")
        dbg["pools"] = nc.dram_tensor("dbg_pools", [128, 5], dt.float32,
                                      kind="ExternalOutput")

    with tile.TileContext(nc) as tc:
        with tc.tile_pool(name="const", bufs=1) as cst, \
             tc.tile_pool(name="streams", bufs=1) as stp, \
             tc.tile_pool(name="big", bufs=1) as bigp, \
             tc.tile_pool(name="gath", bufs=3) as gp, \
             tc.tile_pool(name="sti", bufs=6) as sti, \
             tc.tile_pool(name="small", bufs=4) as smp, \
             tc.tile_pool(name="stats", bufs=1) as statp, \
             tc.tile_pool(name="dram", bufs=2, space="DRAM") as dramp:

            htab_loc = dramp.tile([NLOC, 2 * D], dt.float8e4,
                                  tag="htab_loc", name="htab_loc")
            htab = dramp.tile([NLOC * NCORES, 2 * D], dt.float8e4, tag="htab",
                              name="htab", addr_space="Shared")

            iota_t = cst.tile([128, WW], dt.bfloat16)
            ones_t = cst.tile([128, 1], dt.bfloat16)
            ident_t = cst.tile([128, 128], dt.bfloat16)
            wnode_t = cst.tile([128, NBLK], dt.float32)
            nc.sync.dma_start(out=iota_t[:], in_=iota_c[:])
            nc.sync.dma_start(out=ones_t[:], in_=ones_c[:])
            nc.sync.dma_start(out=ident_t[:], in_=ident_c[:])
            nc.sync.dma_start(out=wnode_t[:], in_=w_node[:])
            wt = {}
            for e in range(2):
                for l in range(2):
                    wt[e, l] = dict(
                        w1=cst.tile([128, H], dt.bfloat16, tag=f"w1_{e}{l}",
                                    name=f"w1t{e}{l}"),
                        w2a=cst.tile([128, 128], dt.bfloat16, tag=f"w2a_{e}{l}",
                                     name=f"w2at{e}{l}"),
                        w2b=cst.tile([128, 128], dt.bfloat16, tag=f"w2b_{e}{l}",
                                     name=f"w2bt{e}{l}"),
                        g1=cst.tile([128, 2], dt.float32, tag=f"g1_{e}{l}",
                                    name=f"g1t{e}{l}"),
                        b1=cst.tile([128, 2], dt.float32, tag=f"b1_{e}{l}",
                                    name=f"b1t{e}{l}"),
                        g2=cst.tile([128, 1], dt.float32, tag=f"g2_{e}{l}",
                                    name=f"g2t{e}{l}"),
                        b2=cst.tile([128, 1], dt.float32, tag=f"b2_{e}{l}",
                                    name=f"b2t{e}{l}"),
                    )
                    nc.sync.dma_start(out=wt[e, l]["w1"][:], in_=W1[e, l][:])
                    nc.sync.dma_start(out=wt[e, l]["w2a"][:], in_=W2[e, l][:128, :])
                    nc.sync.dma_start(out=wt[e, l]["w2b"][:], in_=W2[e, l][128:, :])
                    nc.sync.dma_start(out=wt[e, l]["g1"][:], in_=G1[e, l][:])
                    nc.sync.dma_start(out=wt[e, l]["b1"][:], in_=B1[e, l][:])
                    nc.sync.dma_start(out=wt[e, l]["g2"][:], in_=G2[e, l][:])
                    nc.sync.dma_start(out=wt[e, l]["b2"][:], in_=B2[e, l][:])

            # big activations
            zT = [bigp.tile([128, NLOC], dt.bfloat16, tag=f"zT{e}",
                            name=f"zT{e}") for e in range(2)]
            hT = [bigp.tile([128, NLOC], dt.bfloat16, tag=f"hT{e}",
                            name=f"hTe{e}") for e in range(2)]
            z2T = [None, None]
            # per-(layer,enc) BN1 partial stats filled during the edge phase
            fusedA = {l: {"ysq": {}, "zs": {}, "arr": {}} for l in range(2)}
            for l in range(2):
                for e in range(2):
                    fusedA[l]["ysq"][e] = statp.tile(
                        [128, 2 * NGRP], dt.float32,
                        tag=f"fysq{l}{e}", name=f"fysq{l}{e}")
                    fusedA[l]["zs"][e] = statp.tile(
                        [128, WIN], dt.float32,
                        tag=f"fzs{l}{e}", name=f"fzs{l}{e}")
            pools = statp.tile([128, 4], dt.float32)  # cols: e0l0,e0l1,e1l0,e1l1
            wcos_acc = statp.tile([128, 1], dt.float32)
            nc.vector.memset(pools[:], 0.0)
            nc.vector.memset(wcos_acc[:], 0.0)

            def all_reduce_stats(stats_tile, ncols):
                """DMA stats [128, ncols] -> AR -> return SBUF tile with result."""
                ari = dramp.tile([128, 8], dt.float32, tag="arin")
                aro = dramp.tile([128, 8], dt.float32, tag="arout",
                                 addr_space="Shared")
                nc.sync.dma_start(out=ari[:, :ncols], in_=stats_tile[:, :ncols])
                if ncols < 8:
                    zpad = smp.tile([128, 8], dt.float32, tag="zpad")
                    nc.vector.memset(zpad[:], 0.0)
                    nc.sync.dma_start(out=ari[:, ncols:], in_=zpad[:, ncols:])
                nc.gpsimd.collective_compute(
                    "AllReduce", mybir.AluOpType.add, replica_groups=rg,
                    ins=[ari[:].opt()], outs=[aro[:].opt()])
                res = smp.tile([128, 8], dt.float32, tag="arres")
                nc.sync.dma_start(out=res[:, :ncols], in_=aro[:, :ncols])
                return res

            # ---------------- edge phase ----------------
            def edge_phase(layer):
                """Produces zT[0], zT[1] = own + agg for both encoders."""
                ctx = nc.named_scope(f"edge{layer}"); ctx.__enter__()
                sc = sched[layer]
                T = sc["T"]
                win_of_tile = sc["win_of_tile"]
                instrs = sc["instrs"]
                table = xf_tab if layer == 0 else htab
                nrow = N if layer == 0 else NLOC * NCORES
                maxnt = max(nt for _, nt, _ in instrs)
                first_of_win = {}
                last_of_win = {}
                tile_instr = {}
                for gi, (t0, nt, b) in enumerate(instrs):
                    for t in range(t0, t0 + nt):
                        tile_instr[t] = gi
                for t in range(T):
                    w = int(win_of_tile[t])
                    first_of_win.setdefault(w, t)
                    last_of_win[w] = t
                gbufs = {}
                with tc.tile_pool(name=f"psum_e{layer}", bufs=1, space="PSUM") as pp, \
                     tc.tile_pool(name=f"psum_a{layer}", bufs=1, space="PSUM") as ppA, \
                     tc.tile_pool(name=f"estr{layer}", bufs=1) as estp, \
                     tc.tile_pool(name=f"gath{layer}", bufs=3) as gp, \
                     tc.tile_pool(name=f"own{layer}", bufs=4) as ownp:
                    idx_sb = estp.tile([128, T * 8], dt.int16, tag="idx")
                    nc.sync.dma_start(out=idx_sb[:], in_=idx_in[layer][:])
                    dst_sb = estp.tile([128, T], dt.float32, tag="dst")
                    nc.sync.dma_start(out=dst_sb[:], in_=dst_in[layer][:])

                    def issue_gather(gi):
                        t0, nt, b = instrs[gi]
                        b1 = min((b + 1) * 32768, nrow)
                        gb = gp.tile([128, maxnt * 2 * D], dt.float8e4,
                                     tag=f"g{gi % 3}", name=f"g{gi % 3}")
                        nc.gpsimd.dma_gather(
                            out_ap=gb[:, :nt * 2 * D].rearrange(
                                "p (t f) -> p t f", f=2 * D),
                            in_ap=table[b * 32768:b1, :],
                            idxs_ap=idx_sb[:, t0 * 8:(t0 + nt) * 8],
                            num_idxs=nt * 128,
                            num_idxs_reg=nt * 128,
                            elem_size=2 * D)
                        gbufs[gi] = gb

                    owns = {}

                    def issue_own(w):
                        cols = slice(w * WW, (w + 1) * WW)
                        if layer == 0:
                            o1 = ownp.tile([128, WW], dt.bfloat16, tag="own1")
                            o2 = ownp.tile([128, WW], dt.bfloat16, tag="own2")
                            nc.sync.dma_start(out=o1[:], in_=xT_own[:, cols])
                            nc.sync.dma_start(out=o2[:], in_=fT_own[:, cols])
                            owns[w] = (o1[:], o2[:])
                        else:
                            owns[w] = (hT[0][:, cols], hT[1][:, cols])

                    def passA_group(gidx):
                        gsz = min(BLK_GRP, NBLK - gidx * BLK_GRP)
                        ncols = gsz * 128
                        c0 = gidx * BLK_GRP * 128
                        for e in (1, 0):
                            p = wt[e, layer]
                            for h in range(2):
                                psy = ppA.tile([128, 512], dt.float32,
                                               space="PSUM", tag=f"psyA{h}",
                                               name=f"psyA{h}")
                                nc.tensor.matmul(
                                    out=psy[:, :ncols],
                                    lhsT=p["w1"][:, h * 128:(h + 1) * 128],
                                    rhs=zT[e][:, c0:c0 + ncols],
                                    start=True, stop=True)
                                deadA = smp.tile([128, 512], dt.bfloat16,
                                                 tag="deadA", name="deadA")
                                nc.scalar.activation(
                                    out=deadA[:, :ncols], in_=psy[:, :ncols],
                                    func=mybir.ActivationFunctionType.Square,
                                    accum_out=fusedA[layer]["ysq"][e][
                                        :, 2 * gidx + h:2 * gidx + h + 1])

                    issue_own(0)
                    next_g = 0
                    psum = {}
                    for t in range(T):
                        gi = tile_instr[t]
                        while next_g <= min(gi + 2, len(instrs) - 1):
                            issue_gather(next_g)
                            next_g += 1
                        w = int(win_of_tile[t])
                        first = (t == first_of_win[w])
                        last = (t == last_of_win[w])
                        if first:
                            if w + 1 < WIN:
                                issue_own(w + 1)
                            psA = pp.tile([128, WW], dt.float32, space="PSUM",
                                          tag=f"psA{w % 3}")
                            psB = pp.tile([128, WW], dt.float32, space="PSUM",
                                          tag=f"psB{w % 3}")
                            psum[w] = (psA, psB)
                            o1, o2 = owns.pop(w)
                            nc.tensor.matmul(out=psA[:], lhsT=ident_t[:],
                                             rhs=o1, start=True, stop=False)
                            nc.tensor.matmul(out=psB[:], lhsT=ident_t[:],
                                             rhs=o2, start=True, stop=False)
                        psA, psB = psum[w]
                        Sf = sti.tile([128, WW], dt.bfloat16, tag="Sf",
                                      name="Sf")
                        nc.vector.tensor_scalar(
                            out=Sf[:], in0=iota_t[:],
                            scalar1=dst_sb[:, t:t + 1], scalar2=None,
                            op0=mybir.AluOpType.is_equal)
                        gb = gbufs[gi]
                        j = t - instrs[gi][0]
                        nc.tensor.matmul(out=psA[:],
                                         lhsT=gb[:, j * 2 * D:j * 2 * D + D],
                                         rhs=Sf[:], start=False, stop=last)
                        nc.tensor.matmul(out=psB[:],
                                         lhsT=gb[:, j * 2 * D + D:(j + 1) * 2 * D],
                                         rhs=Sf[:], start=False, stop=last)
                        if last:
                            del psum[w]
                            cols = slice(w * WW, (w + 1) * WW)
                            nc.scalar.activation(
                                out=zT[0][:, cols], in_=psA[:],
                                func=mybir.ActivationFunctionType.Copy,
                                accum_out=fusedA[layer]["zs"][0][:, w:w + 1])
                            nc.scalar.activation(
                                out=zT[1][:, cols], in_=psB[:],
                                func=mybir.ActivationFunctionType.Copy,
                                accum_out=fusedA[layer]["zs"][1][:, w:w + 1])
                            if w % 2 == 1 or w == WIN - 1:
                                passA_group(w // 2)
                    # final stat reduces + AR issue for both encoders
                    for e in (1, 0):
                        st = statp.tile([128, 4], dt.float32, tag="bnstat",
                                        name="bnstat")
                        for h in range(2):
                            nc.vector.tensor_reduce(
                                out=st[:, h:h + 1],
                                in_=fusedA[layer]["ysq"][e][:, h:2 * NGRP:2],
                                axis=mybir.AxisListType.X,
                                op=mybir.AluOpType.add)
                        nc.vector.tensor_reduce(
                            out=st[:, 2:3], in_=fusedA[layer]["zs"][e][:],
                            axis=mybir.AxisListType.X, op=mybir.AluOpType.add)
                        fusedA[layer]["arr"][e] = all_reduce_stats(st, 3)

                ctx.__exit__(None, None, None)

            # ---------------- node phase ----------------
            def node_phase(e, layer, write_table, pp, nsp, trp, ptp):
                p = wt[e, layer]
                z = zT[e]
                with contextlib.nullcontext():
                    arr = fusedA[layer]["arr"][e]
                    yield
                    # BN1: mean via W1.T @ zsum (tiny matmuls post-AR)
                    zsum_bf = smp.tile([128, 1], dt.bfloat16, tag="zsbf")
                    nc.vector.tensor_copy(out=zsum_bf[:], in_=arr[:, 2:3])
                    psyS = pp.tile([128, 512], dt.float32, space="PSUM",
                                   tag="psz")
                    for h in range(2):
                        nc.tensor.matmul(out=psyS[:, h:h + 1],
                                         lhsT=p["w1"][:, h * 128:(h + 1) * 128],
                                         rhs=zsum_bf[:], start=True, stop=True)
                    ysums = smp.tile([128, 2], dt.float32, tag="ysums")
                    nc.vector.tensor_copy(out=ysums[:], in_=psyS[:, 0:2])
                    # affine: s = g/sqrt(var+eps), t = b - mean*s  (per half)
                    s1 = smp.tile([128, 2], dt.float32, tag="s1")
                    t1 = smp.tile([128, 2], dt.float32, tag="t1")
                    mean = smp.tile([128, 2], dt.float32, tag="mean")
                    var = smp.tile([128, 2], dt.float32, tag="var")
                    nc.vector.tensor_scalar_mul(mean[:], ysums[:], 1.0 / N)
                    nc.vector.tensor_scalar_mul(var[:], arr[:, 0:2], 1.0 / N)
                    msq = smp.tile([128, 2], dt.float32, tag="msq")
                    nc.vector.tensor_mul(msq[:], mean[:], mean[:])
                    nc.vector.tensor_sub(var[:], var[:], msq[:])
                    nc.vector.tensor_scalar_add(var[:], var[:], BN_EPS)
                    nc.vector.reciprocal(var[:], var[:])
                    nc.scalar.activation(out=var[:], in_=var[:],
                                         func=mybir.ActivationFunctionType.Sqrt)
                    nc.vector.tensor_mul(s1[:], p["g1"][:], var[:])
                    nc.vector.tensor_mul(t1[:], mean[:], s1[:])
                    nc.vector.tensor_sub(t1[:], p["b1"][:], t1[:])

                    # ---- pass B: recompute y, BN1+relu, W2 -> z2, stats2 ----
                    z2sum = nsp.tile([128, NGRP + 1], dt.float32, tag="z2sum")
                    z2sq = nsp.tile([128, NGRP + 1], dt.float32, tag="z2sq")
                    for g in range(NGRP):
                        gsz = min(BLK_GRP, NBLK - g * BLK_GRP)
                        ncols = gsz * 128
                        c0 = g * BLK_GRP * 128
                        zsl = z[:, c0:c0 + ncols]
                        psz = pp.tile([128, 512], dt.float32, space="PSUM",
                                      tag="psz")
                        for h in range(2):
                            psy = pp.tile([128, 512], dt.float32, space="PSUM",
                                          tag=f"psy{h}")
                            nc.tensor.matmul(out=psy[:, :ncols],
                                             lhsT=p["w1"][:, h * 128:(h + 1) * 128],
                                             rhs=zsl, start=True, stop=True)
                            hm = nsp.tile([128, 512], dt.bfloat16, tag=f"hm{h}")
                            nc.scalar.activation(
                                out=hm[:, :ncols], in_=psy[:, :ncols],
                                func=mybir.ActivationFunctionType.Relu,
                                bias=t1[:, h:h + 1], scale=s1[:, h:h + 1])
                            nc.tensor.matmul(out=psz[:, :ncols],
                                             lhsT=p["w2a" if h == 0 else "w2b"][:],
                                             rhs=hm[:, :ncols],
                                             start=(h == 0), stop=(h == 1))
                        nc.vector.tensor_copy(out=z2T[e][:, c0:c0 + ncols],
                                              in_=psz[:, :ncols])
                        # exclude padded phantom nodes from stats
                        vcols = min(ncols, NSHARD - c0)
                        dead = nsp.tile([128, 512], dt.bfloat16, tag="dead")
                        nc.scalar.activation(
                            out=dead[:, :vcols], in_=psz[:, :vcols],
                            func=mybir.ActivationFunctionType.Square,
                            accum_out=z2sq[:, g:g + 1])
                        nc.vector.tensor_reduce(
                            out=z2sum[:, g:g + 1], in_=psz[:, :vcols],
                            axis=mybir.AxisListType.X, op=mybir.AluOpType.add)
                    if NLOC > NSHARD:
                        nc.vector.memset(z2T[e][:, NSHARD:NLOC], 0.0)
                    st2 = statp.tile([128, 4], dt.float32, tag="bnstat2")
                    nc.vector.tensor_reduce(out=st2[:, 0:1], in_=z2sq[:, :NGRP],
                                            axis=mybir.AxisListType.X,
                                            op=mybir.AluOpType.add)
                    nc.vector.tensor_reduce(out=st2[:, 1:2], in_=z2sum[:, :NGRP],
                                            axis=mybir.AxisListType.X,
                                            op=mybir.AluOpType.add)
                    arr2 = all_reduce_stats(st2, 2)
                    yield
                    s2 = smp.tile([128, 1], dt.float32, tag="s2")
                    t2 = smp.tile([128, 1], dt.float32, tag="t2")
                    mean2 = smp.tile([128, 1], dt.float32, tag="mean2")
                    var2 = smp.tile([128, 1], dt.float32, tag="var2")
                    nc.vector.tensor_scalar_mul(mean2[:], arr2[:, 1:2], 1.0 / N)
                    nc.vector.tensor_scalar_mul(var2[:], arr2[:, 0:1], 1.0 / N)
                    msq2 = smp.tile([128, 1], dt.float32, tag="msq2")
                    nc.vector.tensor_mul(msq2[:], mean2[:], mean2[:])
                    nc.vector.tensor_sub(var2[:], var2[:], msq2[:])
                    nc.vector.tensor_scalar_add(var2[:], var2[:], BN_EPS)
                    nc.vector.reciprocal(var2[:], var2[:])
                    nc.scalar.activation(out=var2[:], in_=var2[:],
                                         func=mybir.ActivationFunctionType.Sqrt)
                    nc.vector.tensor_mul(s2[:], p["g2"][:], var2[:])
                    nc.vector.tensor_mul(t2[:], mean2[:], s2[:])
                    nc.vector.tensor_sub(t2[:], p["b2"][:], t2[:])

                    # ---- pass C: h = relu(BN2(z2)), pools, optional table ----
                    poolstage = nsp.tile([128, NGRP + 1], dt.float32, tag="pst")

                    def passC_groups(g0_, g1_):
                        for g in range(g0_, g1_):
                            gsz = min(BLK_GRP, NBLK - g * BLK_GRP)
                            ncols = gsz * 128
                            c0 = g * BLK_GRP * 128
                            nc.scalar.activation(
                                out=hT[e][:, c0:c0 + ncols],
                                in_=z2T[e][:, c0:c0 + ncols],
                                func=mybir.ActivationFunctionType.Relu,
                                bias=t2[:, 0:1], scale=s2[:, 0:1],
                                accum_out=poolstage[:, g:g + 1])
                            if write_table:
                                stg = trp.tile([128, 512], dt.float8e4,
                                               tag="stg")
                                for jj in range(gsz):
                                    blk = g * BLK_GRP + jj
                                    pst = ptp.tile([128, 128], dt.bfloat16,
                                                   space="PSUM", tag="pt")
                                    nc.tensor.transpose(
                                        out=pst[:],
                                        in_=hT[e][:, blk * 128:(blk + 1) * 128],
                                        identity=ident_t[:])
                                    nc.vector.tensor_copy(
                                        out=stg[:, jj * 128:(jj + 1) * 128],
                                        in_=pst[:])
                                nc.sync.dma_start(
                                    out=htab_loc[
                                        g * 512:g * 512 + gsz * 128,
                                        e * D:(e + 1) * D].rearrange(
                                        "(c p) f -> p c f", p=128),
                                    in_=stg[:, :gsz * 128].rearrange(
                                        "p (c f) -> p c f", f=128))

                    passC_groups(0, NGRP)

                    # pools: pad correction (z2 pad cols are 0 -> h=relu(t2))
                    npad = NLOC - NSHARD
                    relut2 = smp.tile([128, 1], dt.float32, tag="relut2")
                    nc.vector.tensor_scalar_max(relut2[:], t2[:], 0.0)
                    nc.vector.tensor_scalar_mul(relut2[:], relut2[:],
                                                -float(npad))
                    pcol = pools[:, 2 * e + layer:2 * e + layer + 1]
                    nc.vector.tensor_reduce(out=pcol, in_=poolstage[:, :NGRP],
                                            axis=mybir.AxisListType.X,
                                            op=mybir.AluOpType.add)
                    nc.vector.tensor_add(out=pcol, in0=pcol, in1=relut2[:])
                    if NLOC > NSHARD:
                        nc.vector.memset(hT[e][:, NSHARD:NLOC], 0.0)

            def run_node_layer(layer, write_table):
                with tc.tile_pool(name=f"psum_n{layer}", bufs=2,
                                  space="PSUM") as pp, \
                     tc.tile_pool(name=f"nstage{layer}", bufs=3) as nsp, \
                     tc.tile_pool(name=f"zz{layer}", bufs=1) as zzp, \
                     tc.tile_pool(name=f"tr{layer}", bufs=3) as trp, \
                     tc.tile_pool(name=f"ptr{layer}", bufs=2,
                                  space="PSUM") as ptp:
                    for e in range(2):
                        z2T[e] = zzp.tile([128, NLOC], dt.bfloat16,
                                          tag=f"z2T{e}", name=f"z2T{e}_{layer}")
                    g1 = node_phase(1, layer, write_table, pp, nsp, trp, ptp)
                    g0 = node_phase(0, layer, write_table, pp, nsp, trp, ptp)
                    next(g1); next(g0)
                    next(g1); next(g0)
                    for _ in g1:
                        pass
                    for _ in g0:
                        pass
                    if write_table:
                        nc.gpsimd.collective_compute(
                            "AllGather", mybir.AluOpType.bypass,
                            replica_groups=rg,
                            ins=[htab_loc[:].opt()], outs=[htab[:].opt()])

            # ---------------- run the pipeline ----------------
            edge_phase(0)
            if DEBUG:
                nc.sync.dma_start(out=dbg["zt0"][:], in_=zT[0][:])
                nc.sync.dma_start(out=dbg["zt1"][:], in_=zT[1][:])
            run_node_layer(0, write_table=True)
            if DEBUG:
                nc.sync.dma_start(out=dbg["h0"][:], in_=hT[0][:])
                nc.sync.dma_start(out=dbg["h1"][:], in_=hT[1][:])
            edge_phase(1)
            if DEBUG:
                nc.sync.dma_start(out=dbg["zl0"][:], in_=zT[0][:])
                nc.sync.dma_start(out=dbg["zl1"][:], in_=zT[1][:])
            run_node_layer(1, write_table=False)
            if DEBUG:
                nc.sync.dma_start(out=dbg["hf0"][:], in_=hT[0][:])
                nc.sync.dma_start(out=dbg["hf1"][:], in_=hT[1][:])

            # ---------------- loss ----------------
            with tc.tile_pool(name="psum_l", bufs=2, space="PSUM") as plp, \
                 tc.tile_pool(name="lstage", bufs=3) as lsp:
                A = lsp.tile([128, NBLK], dt.float32, tag="A")
                B = lsp.tile([128, NBLK], dt.float32, tag="B")
                C = lsp.tile([128, NBLK], dt.float32, tag="C")
                for g in range(NGRP):
                    gsz = min(BLK_GRP, NBLK - g * BLK_GRP)
                    ncols = gsz * 128
                    c0 = g * BLK_GRP * 128
                    u = lsp.tile([128, 512], dt.bfloat16, tag="u")
                    q1 = lsp.tile([128, 512], dt.bfloat16, tag="q1")
                    q2 = lsp.tile([128, 512], dt.bfloat16, tag="q2")
                    nc.vector.tensor_mul(u[:, :ncols], hT[0][:, c0:c0 + ncols],
                                         hT[1][:, c0:c0 + ncols])
                    nc.vector.tensor_mul(q1[:, :ncols], hT[0][:, c0:c0 + ncols],
                                         hT[0][:, c0:c0 + ncols])
                    nc.vector.tensor_mul(q2[:, :ncols], hT[1][:, c0:c0 + ncols],
                                         hT[1][:, c0:c0 + ncols])
                    pa = plp.tile([128, BLK_GRP], dt.float32, space="PSUM",
                                  tag="pa")
                    pb = plp.tile([128, BLK_GRP], dt.float32, space="PSUM",
                                  tag="pb")
                    pc2 = plp.tile([128, BLK_GRP], dt.float32, space="PSUM",
                                   tag="pc2")
                    for jj in range(gsz):
                        sl = slice(jj * 128, (jj + 1) * 128)
                        nc.tensor.matmul(out=pa[:, jj:jj + 1], lhsT=u[:, sl],
                                         rhs=ones_t[:], start=True, stop=True)
                        nc.tensor.matmul(out=pb[:, jj:jj + 1], lhsT=q1[:, sl],
                                         rhs=ones_t[:], start=True, stop=True)
                        nc.tensor.matmul(out=pc2[:, jj:jj + 1], lhsT=q2[:, sl],
                                         rhs=ones_t[:], start=True, stop=True)
                    gs = slice(g * BLK_GRP, g * BLK_GRP + gsz)
                    nc.vector.tensor_copy(out=A[:, gs], in_=pa[:, :gsz])
                    nc.vector.tensor_copy(out=B[:, gs], in_=pb[:, :gsz])
                    nc.vector.tensor_copy(out=C[:, gs], in_=pc2[:, :gsz])
                # wcos = w * a / sqrt(b*c)
                BC = lsp.tile([128, NBLK], dt.float32, tag="BC")
                nc.vector.tensor_mul(BC[:], B[:], C[:])
                nc.vector.tensor_scalar_max(BC[:], BC[:], 1e-24)
                nc.vector.reciprocal(BC[:], BC[:])
                nc.scalar.activation(out=BC[:], in_=BC[:],
                                     func=mybir.ActivationFunctionType.Sqrt)
                nc.vector.tensor_mul(BC[:], BC[:], A[:])
                nc.vector.tensor_mul(BC[:], BC[:], wnode_t[:])
                nc.vector.tensor_reduce(out=wcos_acc[:], in_=BC[:],
                                        axis=mybir.AxisListType.X,
                                        op=mybir.AluOpType.add)

                if DEBUG:
                    nc.sync.dma_start(out=dbg["pools"][:, 0:4], in_=pools[:])
                    nc.sync.dma_start(out=dbg["pools"][:, 4:5], in_=wcos_acc[:])
                fin = statp.tile([128, 5], dt.float32, tag="fin")
                nc.vector.tensor_copy(out=fin[:, 0:4], in_=pools[:])
                nc.vector.tensor_copy(out=fin[:, 4:5], in_=wcos_acc[:])
                arr_p = all_reduce_stats(fin, 5)
                arr_w = arr_p[:, 4:5]
                stack = lsp.tile([128, 4], dt.float32, tag="stack")
                t_a = lsp.tile([128, 2], dt.float32, tag="t_a")
                t_b = lsp.tile([128, 2], dt.float32, tag="t_b")
                t_c = lsp.tile([128, 2], dt.float32, tag="t_c")
                nc.vector.tensor_mul(t_a[:], arr_p[:, 0:2], arr_p[:, 2:4])
                nc.vector.tensor_mul(t_b[:], arr_p[:, 0:2], arr_p[:, 0:2])
                nc.vector.tensor_mul(t_c[:], arr_p[:, 2:4], arr_p[:, 2:4])
                nc.vector.tensor_reduce(out=stack[:, 0:1], in_=t_a[:],
                                        axis=mybir.AxisListType.X,
                                        op=mybir.AluOpType.add)
                nc.vector.tensor_reduce(out=stack[:, 1:2], in_=t_b[:],
                                        axis=mybir.AxisListType.X,
                                        op=mybir.AluOpType.add)
                nc.vector.tensor_reduce(out=stack[:, 2:3], in_=t_c[:],
                                        axis=mybir.AxisListType.X,
                                        op=mybir.AluOpType.add)
                nc.vector.tensor_copy(out=stack[:, 3:4], in_=arr_w)
                stack_bf = lsp.tile([128, 4], dt.float32, tag="stackbf")
                nc.vector.tensor_copy(out=stack_bf[:], in_=stack[:])
                pfin = plp.tile([1, 4], dt.float32, space="PSUM", tag="pfin")
                ones_f = lsp.tile([128, 1], dt.float32, tag="onesf")
                nc.vector.memset(ones_f[:], 1.0)
                for jj in range(4):
                    nc.tensor.matmul(out=pfin[:, jj:jj + 1],
                                     lhsT=stack_bf[:, jj:jj + 1],
                                     rhs=ones_f[:], start=True, stop=True)
                # loss = 0.5*(1 - wcos/Mn) + 0.5*(1 - dot/sqrt(n1*n2))
                sc = lsp.tile([1, 4], dt.float32, tag="sc")
                nc.vector.tensor_copy(out=sc[:], in_=pfin[:])
                g2v = lsp.tile([1, 1], dt.float32, tag="g2v")
                nc.vector.tensor_mul(g2v[:], sc[:, 1:2], sc[:, 2:3])
                nc.vector.tensor_scalar_max(g2v[:], g2v[:], 1e-24)
                nc.vector.reciprocal(g2v[:], g2v[:])
                nc.scalar.activation(out=g2v[:], in_=g2v[:],
                                     func=mybir.ActivationFunctionType.Sqrt)
                nc.vector.tensor_mul(g2v[:], g2v[:], sc[:, 0:1])  # cos_g
                res = lsp.tile([1, 1], dt.float32, tag="res")
                nc.vector.tensor_scalar_mul(res[:], sc[:, 3:4],
                                            -0.5 / float(Mn))
                nc.vector.tensor_scalar_add(res[:], res[:], 1.0)
                half = lsp.tile([1, 1], dt.float32, tag="half")
                nc.vector.tensor_scalar_mul(half[:], g2v[:], 0.5)
                nc.vector.tensor_sub(res[:], res[:], half[:])
                nc.sync.dma_start(out=out_loss[:], in_=res[:])

    nc.compile()
    return nc


# ---------------- public entry ----------------
_CACHE = {}


def prepare(feat, mask_token, oW1, oW2, og1, ob1, og2, ob2,
            tW1, tW2, tg1, tb1, tg2, tb2, src, dst, mask_nodes):
    feat = np.asarray(feat, np.float32)
    mask_token = np.asarray(mask_token, np.float32)
    src = np.asarray(src, np.int32)
    dst = np.asarray(dst, np.int32)
    mask_nodes = np.asarray(mask_nodes, np.int32)

    sched = preprocess(src, dst)

    key = (sched[0]["T"], sched[1]["T"], len(mask_nodes))
    if key not in _CACHE:
        _CACHE[key] = build(sched, len(mask_nodes))
    nc = _CACHE[key]

    # host tensors
    x = feat.copy()
    x[mask_nodes] = np.broadcast_to(mask_token, (len(mask_nodes), D))
    xf = np.concatenate([x, feat], axis=1).astype(f8)   # [N, 256] fp8
    iota_c = np.broadcast_to(np.arange(WW, dtype=np.float32),
                             (128, WW)).astype(bf16)
    ones_c = np.ones((128, 1), bf16)
    ident_c = np.eye(128, dtype=np.float32).astype(bf16)
    is_m = np.bincount(mask_nodes, minlength=N).astype(np.float32)

    enc_params = [
        (oW1, oW2, og1, ob1, og2, ob2),   # e=0 online (masked input)
        (tW1, tW2, tg1, tb1, tg2, tb2),   # e=1 target
    ]

    common = {"xf_tab": xf, "iota_c": iota_c, "ones_c": ones_c,
              "ident_c": ident_c}
    for e in range(2):
        w1, w2, g1, b1, g2, b2 = enc_params[e]
        for l in range(2):
            common[f"w1_{e}{l}"] = np.asarray(w1[l], np.float32).astype(bf16)
            common[f"w2_{e}{l}"] = np.asarray(w2[l], np.float32).astype(bf16)
            common[f"g1_{e}{l}"] = np.asarray(g1[l], np.float32).reshape(2, 128).T.copy()
            common[f"b1_{e}{l}"] = np.asarray(b1[l], np.float32).reshape(2, 128).T.copy()
            common[f"g2_{e}{l}"] = np.asarray(g2[l], np.float32).reshape(128, 1).copy()
            common[f"b2_{e}{l}"] = np.asarray(b2[l], np.float32).reshape(128, 1).copy()

    in_maps = []
    for c in range(NCORES):
        rows = slice(c * NSHARD, (c + 1) * NSHARD)
        xT = np.zeros((128, NLOC), np.float32)
        fT = np.zeros((128, NLOC), np.float32)
        xT[:, :NSHARD] = x[rows].T
        fT[:, :NSHARD] = feat[rows].T
        wn = np.zeros(NLOC, np.float32)
        wn[:NSHARD] = is_m[rows]
        m = dict(common)
        m.update({
            "xT_own": xT.astype(bf16), "fT_own": fT.astype(bf16),
            "idx0": sched[0]["idx16"][c], "idx1": sched[1]["idx16"][c],
            "dst0": sched[0]["dstA"][c], "dst1": sched[1]["dstA"][c],
            "w_node": wn.reshape(NBLK, 128).T.copy(),
        })
        in_maps.append(m)

    return nc, in_maps


def kernel(**inputs):
    nc, in_maps = prepare(**inputs)
    last_err = None
    for attempt in range(3):
        try:
            res = run_bass_kernel_spmd(nc, in_maps, core_ids=list(range(NCORES)))
            loss = res.results[0]["loss"].reshape(())
            return np.float32(loss)
        except Exception as e:  # transient NRT device errors happen; retry
            last_err = e
    raise last_err


# revision 13
# speedup vs baseline: 1.3806x; 1.2208x over previous
"""GraphMAE-style 2-layer GIN loss kernel for one TRN2 chip (8 NeuronCores).

Self-contained: builds + compiles a Bass/Tile SPMD kernel, shards the graph on
the host (dst-partitioned nodes + incident edges), runs on 8 cores via
run_bass_kernel_spmd, returns the scalar loss.

Design (v2, optimized):
  - nodes sharded: core c owns rows [12500c, 12500(c+1)), padded to 12544
  - gather tables hold BOTH encoders' features per row (256B fp8):
    layer 0: [x | feat], layer 1: [h0 | h1]; one batched indirect DMA
    gathers GRP*128 edges per instruction (SWDGE cost is ~1us fixed +
    0.34ns/row, so batching is ~6x cheaper than per-tile gathers)
  - segment_sum: per dst-window of 256 nodes, PSUM accumulates
    gathered_rows.T @ onehot(dst); the node's own contribution is seeded
    into PSUM with an identity-weight matmul (z = own + agg)
  - one-hot built by DVE tensor_scalar is_equal (bf16, 4x perf mode)
  - BN1 stats: sum(y) = W1.T @ sum(z) (tiny matmul after AllReduce);
    sum(y^2) accumulated by ScalarE Square activations during edge phase
  - h tables written transposed (fp8) to a shared DRAM table via a
    4-chunk AllGather overlapped with the transpose writes
  - feat-major activations [128 feat, nodes]; BN affine + relu on ScalarE
"""
import sys, os, contextlib
sys.path.insert(0, '/opt/trn_rl_repo')
import numpy as np
import ml_dtypes

from concourse import bass, bacc, tile, mybir
from concourse.bass_utils import run_bass_kernel_spmd

dt = mybir.dt
bf16 = ml_dtypes.bfloat16
f8 = ml_dtypes.float8_e4m3

# ---------------- problem constants ----------------
N = 100000
E = 800000
D = 128
H = 256
L = 2
NCORES = 8
NSHARD = N // NCORES          # 12500
NLOC = 12544                  # padded to 98*128
WW = 256                      # dst-window width (nodes)
WIN = NLOC // WW              # 49 windows per core
NBLK = NLOC // 128            # 98 node blocks per core
BN_EPS = 1e-5
BLK_GRP = 4                   # node blocks per matmul group (free dim 512)
NGRP = (NBLK + BLK_GRP - 1) // BLK_GRP   # 25


# ---------------- host-side graph preprocessing ----------------
def preprocess(src, dst):
    """Build per-core edge streams + shared schedule, one per layer.

    Stream order: (window-pair, bin, window, src).  Each gather instruction
    covers the tiles of one (pair, bin) run; int16 indices are bin-local.
    """
    src = np.asarray(src).astype(np.int64)
    dst = np.asarray(dst).astype(np.int64)
    core_of = dst // NSHARD
    dst_loc = dst % NSHARD
    win = dst_loc // WW
    dloc = dst_loc % WW
    NPAIR = (WIN + 1) // 2

    out = []
    for layer in range(2):
        if layer == 0:
            row = src
            nrow = N
        else:
            row = (src // NSHARD) * NLOC + src % NSHARD
            nrow = NLOC * NCORES
        nbin = (nrow + 32767) // 32768
        bin_ = row // 32768
        loc16 = (row - bin_ * 32768).astype(np.int64)

        # per-core counts per (win, bin)
        counts = np.zeros((NCORES, WIN, nbin), np.int64)
        sels = []
        for c in range(NCORES):
            sel = np.nonzero(core_of == c)[0]
            order = np.lexsort((src[sel], bin_[sel], win[sel]))
            sel = sel[order]
            sels.append(sel)
            np.add.at(counts[c], (win[sel], bin_[sel]), 1)
        ntile = (counts.max(axis=0) + 127) // 128      # [WIN, nbin]

        # tile stream ordered (pair, bin, win); instruction per (pair, bin)
        win_of_tile = []
        instrs = []            # (t0, nt, bin)
        tile_start = np.zeros((WIN, nbin), np.int64)
        for pr in range(NPAIR):
            ws = [2 * pr] + ([2 * pr + 1] if 2 * pr + 1 < WIN else [])
            for b in range(nbin):
                nt = int(sum(ntile[w, b] for w in ws))
                if nt == 0:
                    continue
                t0 = len(win_of_tile)
                # dma_gather is limited to 1024 indices (8 tiles) per instr
                for o in range(0, nt, 8):
                    instrs.append((t0 + o, min(8, nt - o), b))
                for w in ws:
                    tile_start[w, b] = len(win_of_tile)
                    win_of_tile += [w] * int(ntile[w, b])
        T = len(win_of_tile)
        win_of_tile = np.array(win_of_tile, np.int64)

        idx16 = np.zeros((NCORES, 128, T * 8), np.int16)
        dstA = np.full((NCORES, T * 128), -1.0, np.float32)
        for c in range(NCORES):
            sel = sels[c]
            # rank within (win, bin) group
            ofs = np.concatenate([[0], np.cumsum(counts[c].ravel())])
            key = win[sel] * nbin + bin_[sel]
            pos = tile_start[win[sel], bin_[sel]] * 128 + \
                (np.arange(len(sel)) - ofs[key])
            # idx stream: edge at stream pos i -> [16*k + i%16, i//16]
            rows = (pos % 16).astype(np.int64)
            cols = (pos // 16).astype(np.int64)
            for k in range(8):
                idx16[c, 16 * k + rows, cols] = loc16[sel]
            dstA[c, pos] = dloc[sel]
        dstA = np.ascontiguousarray(
            dstA.reshape(NCORES, T, 128).transpose(0, 2, 1)).astype(np.float32)
        out.append(dict(T=T, win_of_tile=win_of_tile, instrs=instrs,
                        nbin=nbin, idx16=idx16, dstA=dstA))
    return out


# ---------------- kernel builder ----------------
def build(sched, Mn):
    nc = bacc.Bacc("TRN2", target_bir_lowering=False, debug=False,
                   num_devices=NCORES, num_swdge_queues=4)
    rg = [list(range(NCORES))]

    def inp(name, shape, d):
        return nc.dram_tensor(name, shape, d, kind="ExternalInput")

    xf_tab = inp("xf_tab", [N, 2 * D], dt.float8e4)
    xT_own = inp("xT_own", [128, NLOC], dt.bfloat16)
    fT_own = inp("fT_own", [128, NLOC], dt.bfloat16)
    idx_in = [inp(f"idx{l}", [128, sched[l]["T"] * 8], dt.int16)
              for l in range(2)]
    dst_in = [inp(f"dst{l}", [128, sched[l]["T"]], dt.float32)
              for l in range(2)]
    iota_c = inp("iota_c", [128, WW], dt.bfloat16)
    ones_c = inp("ones_c", [128, 1], dt.bfloat16)
    ident_c = inp("ident_c", [128, 128], dt.bfloat16)
    w_node = inp("w_node", [128, NBLK], dt.float32)
    W1 = {}; W2 = {}; G1 = {}; B1 = {}; G2 = {}; B2 = {}
    for e in range(2):
        for l in range(2):
            W1[e, l] = inp(f"w1_{e}{l}", [128, H], dt.bfloat16)
            W2[e, l] = inp(f"w2_{e}{l}", [H, 128], dt.bfloat16)
            G1[e, l] = inp(f"g1_{e}{l}", [128, 2], dt.float32)
            B1[e, l] = inp(f"b1_{e}{l}", [128, 2], dt.float32)
            G2[e, l] = inp(f"g2_{e}{l}", [128, 1], dt.float32)
            B2[e, l] = inp(f"b2_{e}{l}", [128, 1], dt.float32)
    out_loss = nc.dram_tensor("loss", [1, 1], dt.float32, kind="ExternalOutput")
    DEBUG = bool(int(os.environ.get("GIN_DEBUG", "0")))
    dbg = {}
    if DEBUG:
        for nm in ["zt0", "zt1", "h0", "h1", "zl0", "zl1", "hf0", "hf1"]:
            dbg[nm] = nc.dram_tensor(f"dbg_{nm}", [128, NLOC], dt.bfloat16,
                                     kind="ExternalOutput")
        dbg["st"] = nc.dram_tensor("dbg_st", [128, 8], dt.float32,
                                   kind="ExternalOutput")
        dbg["pools"] = nc.dram_tensor("dbg_pools", [128, 5], dt.float32,
                                      kind="ExternalOutput")

    with tile.TileContext(nc) as tc:
        with tc.tile_pool(name="const", bufs=1) as cst, \
             tc.tile_pool(name="streams", bufs=1) as stp, \
             tc.tile_pool(name="big", bufs=1) as bigp, \
             tc.tile_pool(name="gath", bufs=3) as gp, \
             tc.tile_pool(name="sti", bufs=6) as sti, \
             tc.tile_pool(name="small", bufs=4) as smp, \
             tc.tile_pool(name="stats", bufs=1) as statp, \
             tc.tile_pool(name="dram", bufs=2, space="DRAM") as dramp:

            htab_loc = dramp.tile([NLOC, 2 * D], dt.float8e4,
                                  tag="htab_loc", name="htab_loc")
            htab = dramp.tile([NLOC * NCORES, 2 * D], dt.float8e4, tag="htab",
                              name="htab", addr_space="Shared")

            iota_t = cst.tile([128, WW], dt.bfloat16)
            ones_t = cst.tile([128, 1], dt.bfloat16)
            ident_t = cst.tile([128, 128], dt.bfloat16)
            wnode_t = cst.tile([128, NBLK], dt.float32)
            nc.sync.dma_start(out=iota_t[:], in_=iota_c[:])
            nc.sync.dma_start(out=ones_t[:], in_=ones_c[:])
            nc.sync.dma_start(out=ident_t[:], in_=ident_c[:])
            nc.sync.dma_start(out=wnode_t[:], in_=w_node[:])
            wt = {}
            for e in range(2):
                for l in range(2):
                    wt[e, l] = dict(
                        w1=cst.tile([128, H], dt.bfloat16, tag=f"w1_{e}{l}",
                                    name=f"w1t{e}{l}"),
                        w2a=cst.tile([128, 128], dt.bfloat16, tag=f"w2a_{e}{l}",
                                     name=f"w2at{e}{l}"),
                        w2b=cst.tile([128, 128], dt.bfloat16, tag=f"w2b_{e}{l}",
                                     name=f"w2bt{e}{l}"),
                        g1=cst.tile([128, 2], dt.float32, tag=f"g1_{e}{l}",
                                    name=f"g1t{e}{l}"),
                        b1=cst.tile([128, 2], dt.float32, tag=f"b1_{e}{l}",
                                    name=f"b1t{e}{l}"),
                        g2=cst.tile([128, 1], dt.float32, tag=f"g2_{e}{l}",
                                    name=f"g2t{e}{l}"),
                        b2=cst.tile([128, 1], dt.float32, tag=f"b2_{e}{l}",
                                    name=f"b2t{e}{l}"),
                    )
                    nc.sync.dma_start(out=wt[e, l]["w1"][:], in_=W1[e, l][:])
                    nc.sync.dma_start(out=wt[e, l]["w2a"][:], in_=W2[e, l][:128, :])
                    nc.sync.dma_start(out=wt[e, l]["w2b"][:], in_=W2[e, l][128:, :])
                    nc.sync.dma_start(out=wt[e, l]["g1"][:], in_=G1[e, l][:])
                    nc.sync.dma_start(out=wt[e, l]["b1"][:], in_=B1[e, l][:])
                    nc.sync.dma_start(out=wt[e, l]["g2"][:], in_=G2[e, l][:])
                    nc.sync.dma_start(out=wt[e, l]["b2"][:], in_=B2[e, l][:])

            # big activations
            zT = [bigp.tile([128, NLOC], dt.bfloat16, tag=f"zT{e}",
                            name=f"zT{e}") for e in range(2)]
            hT = [bigp.tile([128, NLOC], dt.bfloat16, tag=f"hT{e}",
                            name=f"hTe{e}") for e in range(2)]
            z2T = [None, None]
            # per-(layer,enc) BN1 partial stats filled during the edge phase
            fusedA = {l: {"ysq": {}, "zs": {}, "arr": {}} for l in range(2)}
            for l in range(2):
                for e in range(2):
                    fusedA[l]["ysq"][e] = statp.tile(
                        [128, 2 * NGRP], dt.float32,
                        tag=f"fysq{l}{e}", name=f"fysq{l}{e}")
                    fusedA[l]["zs"][e] = statp.tile(
                        [128, WIN], dt.float32,
                        tag=f"fzs{l}{e}", name=f"fzs{l}{e}")
            pools = statp.tile([128, 4], dt.float32)  # cols: e0l0,e0l1,e1l0,e1l1
            wcos_acc = statp.tile([128, 1], dt.float32)
            nc.vector.memset(pools[:], 0.0)
            nc.vector.memset(wcos_acc[:], 0.0)

            def all_reduce_stats(stats_tile, ncols):
                """DMA stats [128, ncols] -> AR -> return SBUF tile with result."""
                ari = dramp.tile([128, 8], dt.float32, tag="arin")
                aro = dramp.tile([128, 8], dt.float32, tag="arout",
                                 addr_space="Shared")
                nc.sync.dma_start(out=ari[:, :ncols], in_=stats_tile[:, :ncols])
                if ncols < 8:
                    zpad = smp.tile([128, 8], dt.float32, tag="zpad")
                    nc.vector.memset(zpad[:], 0.0)
                    nc.sync.dma_start(out=ari[:, ncols:], in_=zpad[:, ncols:])
                nc.gpsimd.collective_compute(
                    "AllReduce", mybir.AluOpType.add, replica_groups=rg,
                    ins=[ari[:].opt()], outs=[aro[:].opt()])
                res = smp.tile([128, 8], dt.float32, tag="arres")
                nc.sync.dma_start(out=res[:, :ncols], in_=aro[:, :ncols])
                return res

            # ---------------- edge phase ----------------
            def edge_phase(layer):
                """Produces zT[0], zT[1] = own + agg for both encoders."""
                ctx = nc.named_scope(f"edge{layer}"); ctx.__enter__()
                sc = sched[layer]
                T = sc["T"]
                win_of_tile = sc["win_of_tile"]
                instrs = sc["instrs"]
                table = xf_tab if layer == 0 else htab
                nrow = N if layer == 0 else NLOC * NCORES
                maxnt = max(nt for _, nt, _ in instrs)
                first_of_win = {}
                last_of_win = {}
                tile_instr = {}
                for gi, (t0, nt, b) in enumerate(instrs):
                    for t in range(t0, t0 + nt):
                        tile_instr[t] = gi
                for t in range(T):
                    w = int(win_of_tile[t])
                    first_of_win.setdefault(w, t)
                    last_of_win[w] = t
                gbufs = {}
                with tc.tile_pool(name=f"psum_e{layer}", bufs=1, space="PSUM") as pp, \
                     tc.tile_pool(name=f"psum_a{layer}", bufs=1, space="PSUM") as ppA, \
                     tc.tile_pool(name=f"estr{layer}", bufs=1) as estp, \
                     tc.tile_pool(name=f"gath{layer}", bufs=3) as gp, \
                     tc.tile_pool(name=f"own{layer}", bufs=4) as ownp:
                    idx_sb = estp.tile([128, T * 8], dt.int16, tag="idx")
                    nc.sync.dma_start(out=idx_sb[:], in_=idx_in[layer][:])
                    dst_sb = estp.tile([128, T], dt.float32, tag="dst")
                    nc.sync.dma_start(out=dst_sb[:], in_=dst_in[layer][:])

                    def issue_gather(gi):
                        t0, nt, b = instrs[gi]
                        b1 = min((b + 1) * 32768, nrow)
                        gb = gp.tile([128, maxnt * 2 * D], dt.float8e4,
                                     tag=f"g{gi % 3}", name=f"g{gi % 3}")
                        nc.gpsimd.dma_gather(
                            out_ap=gb[:, :nt * 2 * D].rearrange(
                                "p (t f) -> p t f", f=2 * D),
                            in_ap=table[b * 32768:b1, :],
                            idxs_ap=idx_sb[:, t0 * 8:(t0 + nt) * 8],
                            num_idxs=nt * 128,
                            num_idxs_reg=nt * 128,
                            elem_size=2 * D,
                            queue_num=gi % 4)
                        gbufs[gi] = gb

                    owns = {}

                    def issue_own(w):
                        cols = slice(w * WW, (w + 1) * WW)
                        if layer == 0:
                            o1 = ownp.tile([128, WW], dt.bfloat16, tag="own1")
                            o2 = ownp.tile([128, WW], dt.bfloat16, tag="own2")
                            nc.sync.dma_start(out=o1[:], in_=xT_own[:, cols])
                            nc.sync.dma_start(out=o2[:], in_=fT_own[:, cols])
                            owns[w] = (o1[:], o2[:])
                        else:
                            owns[w] = (hT[0][:, cols], hT[1][:, cols])

                    def passA_group(gidx):
                        gsz = min(BLK_GRP, NBLK - gidx * BLK_GRP)
                        ncols = gsz * 128
                        c0 = gidx * BLK_GRP * 128
                        for e in (1, 0):
                            p = wt[e, layer]
                            for h in range(2):
                                psy = ppA.tile([128, 512], dt.float32,
                                               space="PSUM", tag=f"psyA{h}",
                                               name=f"psyA{h}")
                                nc.tensor.matmul(
                                    out=psy[:, :ncols],
                                    lhsT=p["w1"][:, h * 128:(h + 1) * 128],
                                    rhs=zT[e][:, c0:c0 + ncols],
                                    start=True, stop=True)
                                deadA = smp.tile([128, 512], dt.bfloat16,
                                                 tag="deadA", name="deadA")
                                nc.scalar.activation(
                                    out=deadA[:, :ncols], in_=psy[:, :ncols],
                                    func=mybir.ActivationFunctionType.Square,
                                    accum_out=fusedA[layer]["ysq"][e][
                                        :, 2 * gidx + h:2 * gidx + h + 1])

                    issue_own(0)
                    next_g = 0
                    psum = {}
                    for t in range(T):
                        gi = tile_instr[t]
                        while next_g <= min(gi + 2, len(instrs) - 1):
                            issue_gather(next_g)
                            next_g += 1
                        w = int(win_of_tile[t])
                        first = (t == first_of_win[w])
                        last = (t == last_of_win[w])
                        if first:
                            if w + 1 < WIN:
                                issue_own(w + 1)
                            psA = pp.tile([128, WW], dt.float32, space="PSUM",
                                          tag=f"psA{w % 3}")
                            psB = pp.tile([128, WW], dt.float32, space="PSUM",
                                          tag=f"psB{w % 3}")
                            psum[w] = (psA, psB)
                            o1, o2 = owns.pop(w)
                            nc.tensor.matmul(out=psA[:], lhsT=ident_t[:],
                                             rhs=o1, start=True, stop=False)
                            nc.tensor.matmul(out=psB[:], lhsT=ident_t[:],
                                             rhs=o2, start=True, stop=False)
                        psA, psB = psum[w]
                        Sf = sti.tile([128, WW], dt.bfloat16, tag="Sf",
                                      name="Sf")
                        nc.vector.tensor_scalar(
                            out=Sf[:], in0=iota_t[:],
                            scalar1=dst_sb[:, t:t + 1], scalar2=None,
                            op0=mybir.AluOpType.is_equal)
                        gb = gbufs[gi]
                        j = t - instrs[gi][0]
                        nc.tensor.matmul(out=psA[:],
                                         lhsT=gb[:, j * 2 * D:j * 2 * D + D],
                                         rhs=Sf[:], start=False, stop=last)
                        nc.tensor.matmul(out=psB[:],
                                         lhsT=gb[:, j * 2 * D + D:(j + 1) * 2 * D],
                                         rhs=Sf[:], start=False, stop=last)
                        if last:
                            del psum[w]
                            cols = slice(w * WW, (w + 1) * WW)
                            nc.scalar.activation(
                                out=zT[0][:, cols], in_=psA[:],
                                func=mybir.ActivationFunctionType.Copy,
                                accum_out=fusedA[layer]["zs"][0][:, w:w + 1])
                            nc.scalar.activation(
                                out=zT[1][:, cols], in_=psB[:],
                                func=mybir.ActivationFunctionType.Copy,
                                accum_out=fusedA[layer]["zs"][1][:, w:w + 1])
                            if w % 2 == 1 or w == WIN - 1:
                                passA_group(w // 2)
                    # final stat reduces + AR issue for both encoders
                    for e in (1, 0):
                        st = statp.tile([128, 4], dt.float32, tag="bnstat",
                                        name="bnstat")
                        for h in range(2):
                            nc.vector.tensor_reduce(
                                out=st[:, h:h + 1],
                                in_=fusedA[layer]["ysq"][e][:, h:2 * NGRP:2],
                                axis=mybir.AxisListType.X,
                                op=mybir.AluOpType.add)
                        nc.vector.tensor_reduce(
                            out=st[:, 2:3], in_=fusedA[layer]["zs"][e][:],
                            axis=mybir.AxisListType.X, op=mybir.AluOpType.add)
                        fusedA[layer]["arr"][e] = all_reduce_stats(st, 3)

                ctx.__exit__(None, None, None)

            # ---------------- node phase ----------------
            def node_phase(e, layer, write_table, pp, nsp, trp, ptp):
                p = wt[e, layer]
                z = zT[e]
                with contextlib.nullcontext():
                    arr = fusedA[layer]["arr"][e]
                    yield
                    # BN1: mean via W1.T @ zsum (tiny matmuls post-AR)
                    zsum_bf = smp.tile([128, 1], dt.bfloat16, tag="zsbf")
                    nc.vector.tensor_copy(out=zsum_bf[:], in_=arr[:, 2:3])
                    psyS = pp.tile([128, 512], dt.float32, space="PSUM",
                                   tag="psz")
                    for h in range(2):
                        nc.tensor.matmul(out=psyS[:, h:h + 1],
                                         lhsT=p["w1"][:, h * 128:(h + 1) * 128],
                                         rhs=zsum_bf[:], start=True, stop=True)
                    ysums = smp.tile([128, 2], dt.float32, tag="ysums")
                    nc.vector.tensor_copy(out=ysums[:], in_=psyS[:, 0:2])
                    # affine: s = g/sqrt(var+eps), t = b - mean*s  (per half)
                    s1 = smp.tile([128, 2], dt.float32, tag="s1")
                    t1 = smp.tile([128, 2], dt.float32, tag="t1")
                    mean = smp.tile([128, 2], dt.float32, tag="mean")
                    var = smp.tile([128, 2], dt.float32, tag="var")
                    nc.vector.tensor_scalar_mul(mean[:], ysums[:], 1.0 / N)
                    nc.vector.tensor_scalar_mul(var[:], arr[:, 0:2], 1.0 / N)
                    msq = smp.tile([128, 2], dt.float32, tag="msq")
                    nc.vector.tensor_mul(msq[:], mean[:], mean[:])
                    nc.vector.tensor_sub(var[:], var[:], msq[:])
                    nc.vector.tensor_scalar_add(var[:], var[:], BN_EPS)
                    nc.vector.reciprocal(var[:], var[:])
                    nc.scalar.activation(out=var[:], in_=var[:],
                                         func=mybir.ActivationFunctionType.Sqrt)
                    nc.vector.tensor_mul(s1[:], p["g1"][:], var[:])
                    nc.vector.tensor_mul(t1[:], mean[:], s1[:])
                    nc.vector.tensor_sub(t1[:], p["b1"][:], t1[:])

                    # ---- pass B: recompute y, BN1+relu, W2 -> z2, stats2 ----
                    z2sum = nsp.tile([128, NGRP + 1], dt.float32, tag="z2sum")
                    z2sq = nsp.tile([128, NGRP + 1], dt.float32, tag="z2sq")
                    for g in range(NGRP):
                        gsz = min(BLK_GRP, NBLK - g * BLK_GRP)
                        ncols = gsz * 128
                        c0 = g * BLK_GRP * 128
                        zsl = z[:, c0:c0 + ncols]
                        psz = pp.tile([128, 512], dt.float32, space="PSUM",
                                      tag="psz")
                        for h in range(2):
                            psy = pp.tile([128, 512], dt.float32, space="PSUM",
                                          tag=f"psy{h}")
                            nc.tensor.matmul(out=psy[:, :ncols],
                                             lhsT=p["w1"][:, h * 128:(h + 1) * 128],
                                             rhs=zsl, start=True, stop=True)
                            hm = nsp.tile([128, 512], dt.bfloat16, tag=f"hm{h}")
                            nc.scalar.activation(
                                out=hm[:, :ncols], in_=psy[:, :ncols],
                                func=mybir.ActivationFunctionType.Relu,
                                bias=t1[:, h:h + 1], scale=s1[:, h:h + 1])
                            nc.tensor.matmul(out=psz[:, :ncols],
                                             lhsT=p["w2a" if h == 0 else "w2b"][:],
                                             rhs=hm[:, :ncols],
                                             start=(h == 0), stop=(h == 1))
                        nc.vector.tensor_copy(out=z2T[e][:, c0:c0 + ncols],
                                              in_=psz[:, :ncols])
                        # exclude padded phantom nodes from stats
                        vcols = min(ncols, NSHARD - c0)
                        dead = nsp.tile([128, 512], dt.bfloat16, tag="dead")
                        nc.scalar.activation(
                            out=dead[:, :vcols], in_=psz[:, :vcols],
                            func=mybir.ActivationFunctionType.Square,
                            accum_out=z2sq[:, g:g + 1])
                        nc.vector.tensor_reduce(
                            out=z2sum[:, g:g + 1], in_=psz[:, :vcols],
                            axis=mybir.AxisListType.X, op=mybir.AluOpType.add)
                    if NLOC > NSHARD:
                        nc.vector.memset(z2T[e][:, NSHARD:NLOC], 0.0)
                    st2 = statp.tile([128, 4], dt.float32, tag="bnstat2")
                    nc.vector.tensor_reduce(out=st2[:, 0:1], in_=z2sq[:, :NGRP],
                                            axis=mybir.AxisListType.X,
                                            op=mybir.AluOpType.add)
                    nc.vector.tensor_reduce(out=st2[:, 1:2], in_=z2sum[:, :NGRP],
                                            axis=mybir.AxisListType.X,
                                            op=mybir.AluOpType.add)
                    arr2 = all_reduce_stats(st2, 2)
                    yield
                    s2 = smp.tile([128, 1], dt.float32, tag="s2")
                    t2 = smp.tile([128, 1], dt.float32, tag="t2")
                    mean2 = smp.tile([128, 1], dt.float32, tag="mean2")
                    var2 = smp.tile([128, 1], dt.float32, tag="var2")
                    nc.vector.tensor_scalar_mul(mean2[:], arr2[:, 1:2], 1.0 / N)
                    nc.vector.tensor_scalar_mul(var2[:], arr2[:, 0:1], 1.0 / N)
                    msq2 = smp.tile([128, 1], dt.float32, tag="msq2")
                    nc.vector.tensor_mul(msq2[:], mean2[:], mean2[:])
                    nc.vector.tensor_sub(var2[:], var2[:], msq2[:])
                    nc.vector.tensor_scalar_add(var2[:], var2[:], BN_EPS)
                    nc.vector.reciprocal(var2[:], var2[:])
                    nc.scalar.activation(out=var2[:], in_=var2[:],
                                         func=mybir.ActivationFunctionType.Sqrt)
                    nc.vector.tensor_mul(s2[:], p["g2"][:], var2[:])
                    nc.vector.tensor_mul(t2[:], mean2[:], s2[:])
                    nc.vector.tensor_sub(t2[:], p["b2"][:], t2[:])

                    # ---- pass C: h = relu(BN2(z2)), pools, optional table ----
                    poolstage = nsp.tile([128, NGRP + 1], dt.float32, tag="pst")

                    def passC_groups(g0_, g1_):
                        for g in range(g0_, g1_):
                            gsz = min(BLK_GRP, NBLK - g * BLK_GRP)
                            ncols = gsz * 128
                            c0 = g * BLK_GRP * 128
                            nc.scalar.activation(
                                out=hT[e][:, c0:c0 + ncols],
                                in_=z2T[e][:, c0:c0 + ncols],
                                func=mybir.ActivationFunctionType.Relu,
                                bias=t2[:, 0:1], scale=s2[:, 0:1],
                                accum_out=poolstage[:, g:g + 1])
                            if write_table:
                                stg = trp.tile([128, 512], dt.float8e4,
                                               tag="stg")
                                for jj in range(gsz):
                                    blk = g * BLK_GRP + jj
                                    pst = ptp.tile([128, 128], dt.bfloat16,
                                                   space="PSUM", tag="pt")
                                    nc.tensor.transpose(
                                        out=pst[:],
                                        in_=hT[e][:, blk * 128:(blk + 1) * 128],
                                        identity=ident_t[:])
                                    nc.vector.tensor_copy(
                                        out=stg[:, jj * 128:(jj + 1) * 128],
                                        in_=pst[:])
                                nc.sync.dma_start(
                                    out=htab_loc[
                                        g * 512:g * 512 + gsz * 128,
                                        e * D:(e + 1) * D].rearrange(
                                        "(c p) f -> p c f", p=128),
                                    in_=stg[:, :gsz * 128].rearrange(
                                        "p (c f) -> p c f", f=128))

                    passC_groups(0, NGRP)

                    # pools: pad correction (z2 pad cols are 0 -> h=relu(t2))
                    npad = NLOC - NSHARD
                    relut2 = smp.tile([128, 1], dt.float32, tag="relut2")
                    nc.vector.tensor_scalar_max(relut2[:], t2[:], 0.0)
                    nc.vector.tensor_scalar_mul(relut2[:], relut2[:],
                                                -float(npad))
                    pcol = pools[:, 2 * e + layer:2 * e + layer + 1]
                    nc.vector.tensor_reduce(out=pcol, in_=poolstage[:, :NGRP],
                                            axis=mybir.AxisListType.X,
                                            op=mybir.AluOpType.add)
                    nc.vector.tensor_add(out=pcol, in0=pcol, in1=relut2[:])
                    if NLOC > NSHARD:
                        nc.vector.memset(hT[e][:, NSHARD:NLOC], 0.0)

            def run_node_layer(layer, write_table):
                with tc.tile_pool(name=f"psum_n{layer}", bufs=2,
                                  space="PSUM") as pp, \
                     tc.tile_pool(name=f"nstage{layer}", bufs=3) as nsp, \
                     tc.tile_pool(name=f"zz{layer}", bufs=1) as zzp, \
                     tc.tile_pool(name=f"tr{layer}", bufs=3) as trp, \
                     tc.tile_pool(name=f"ptr{layer}", bufs=2,
                                  space="PSUM") as ptp:
                    for e in range(2):
                        z2T[e] = zzp.tile([128, NLOC], dt.bfloat16,
                                          tag=f"z2T{e}", name=f"z2T{e}_{layer}")
                    g1 = node_phase(1, layer, write_table, pp, nsp, trp, ptp)
                    g0 = node_phase(0, layer, write_table, pp, nsp, trp, ptp)
                    next(g1); next(g0)
                    next(g1); next(g0)
                    for _ in g1:
                        pass
                    for _ in g0:
                        pass
                    if write_table:
                        nc.gpsimd.collective_compute(
                            "AllGather", mybir.AluOpType.bypass,
                            replica_groups=rg,
                            ins=[htab_loc[:].opt()], outs=[htab[:].opt()])

            # ---------------- run the pipeline ----------------
            edge_phase(0)
            if DEBUG:
                nc.sync.dma_start(out=dbg["zt0"][:], in_=zT[0][:])
                nc.sync.dma_start(out=dbg["zt1"][:], in_=zT[1][:])
            run_node_layer(0, write_table=True)
            if DEBUG:
                nc.sync.dma_start(out=dbg["h0"][:], in_=hT[0][:])
                nc.sync.dma_start(out=dbg["h1"][:], in_=hT[1][:])
            edge_phase(1)
            if DEBUG:
                nc.sync.dma_start(out=dbg["zl0"][:], in_=zT[0][:])
                nc.sync.dma_start(out=dbg["zl1"][:], in_=zT[1][:])
            run_node_layer(1, write_table=False)
            if DEBUG:
                nc.sync.dma_start(out=dbg["hf0"][:], in_=hT[0][:])
                nc.sync.dma_start(out=dbg["hf1"][:], in_=hT[1][:])

            # ---------------- loss ----------------
            with tc.tile_pool(name="psum_l", bufs=2, space="PSUM") as plp, \
                 tc.tile_pool(name="lstage", bufs=3) as lsp:
                A = lsp.tile([128, NBLK], dt.float32, tag="A")
                B = lsp.tile([128, NBLK], dt.float32, tag="B")
                C = lsp.tile([128, NBLK], dt.float32, tag="C")
                for g in range(NGRP):
                    gsz = min(BLK_GRP, NBLK - g * BLK_GRP)
                    ncols = gsz * 128
                    c0 = g * BLK_GRP * 128
                    u = lsp.tile([128, 512], dt.bfloat16, tag="u")
                    q1 = lsp.tile([128, 512], dt.bfloat16, tag="q1")
                    q2 = lsp.tile([128, 512], dt.bfloat16, tag="q2")
                    nc.vector.tensor_mul(u[:, :ncols], hT[0][:, c0:c0 + ncols],
                                         hT[1][:, c0:c0 + ncols])
                    nc.vector.tensor_mul(q1[:, :ncols], hT[0][:, c0:c0 + ncols],
                                         hT[0][:, c0:c0 + ncols])
                    nc.vector.tensor_mul(q2[:, :ncols], hT[1][:, c0:c0 + ncols],
                                         hT[1][:, c0:c0 + ncols])
                    pa = plp.tile([128, BLK_GRP], dt.float32, space="PSUM",
                                  tag="pa")
                    pb = plp.tile([128, BLK_GRP], dt.float32, space="PSUM",
                                  tag="pb")
                    pc2 = plp.tile([128, BLK_GRP], dt.float32, space="PSUM",
                                   tag="pc2")
                    for jj in range(gsz):
                        sl = slice(jj * 128, (jj + 1) * 128)
                        nc.tensor.matmul(out=pa[:, jj:jj + 1], lhsT=u[:, sl],
                                         rhs=ones_t[:], start=True, stop=True)
                        nc.tensor.matmul(out=pb[:, jj:jj + 1], lhsT=q1[:, sl],
                                         rhs=ones_t[:], start=True, stop=True)
                        nc.tensor.matmul(out=pc2[:, jj:jj + 1], lhsT=q2[:, sl],
                                         rhs=ones_t[:], start=True, stop=True)
                    gs = slice(g * BLK_GRP, g * BLK_GRP + gsz)
                    nc.vector.tensor_copy(out=A[:, gs], in_=pa[:, :gsz])
                    nc.vector.tensor_copy(out=B[:, gs], in_=pb[:, :gsz])
                    nc.vector.tensor_copy(out=C[:, gs], in_=pc2[:, :gsz])
                # wcos = w * a / sqrt(b*c)
                BC = lsp.tile([128, NBLK], dt.float32, tag="BC")
                nc.vector.tensor_mul(BC[:], B[:], C[:])
                nc.vector.tensor_scalar_max(BC[:], BC[:], 1e-24)
                nc.vector.reciprocal(BC[:], BC[:])
                nc.scalar.activation(out=BC[:], in_=BC[:],
                                     func=mybir.ActivationFunctionType.Sqrt)
                nc.vector.tensor_mul(BC[:], BC[:], A[:])
                nc.vector.tensor_mul(BC[:], BC[:], wnode_t[:])
                nc.vector.tensor_reduce(out=wcos_acc[:], in_=BC[:],
                                        axis=mybir.AxisListType.X,
                                        op=mybir.AluOpType.add)

                if DEBUG:
                    nc.sync.dma_start(out=dbg["pools"][:, 0:4], in_=pools[:])
                    nc.sync.dma_start(out=dbg["pools"][:, 4:5], in_=wcos_acc[:])
                fin = statp.tile([128, 5], dt.float32, tag="fin")
                nc.vector.tensor_copy(out=fin[:, 0:4], in_=pools[:])
                nc.vector.tensor_copy(out=fin[:, 4:5], in_=wcos_acc[:])
                arr_p = all_reduce_stats(fin, 5)
                arr_w = arr_p[:, 4:5]
                stack = lsp.tile([128, 4], dt.float32, tag="stack")
                t_a = lsp.tile([128, 2], dt.float32, tag="t_a")
                t_b = lsp.tile([128, 2], dt.float32, tag="t_b")
                t_c = lsp.tile([128, 2], dt.float32, tag="t_c")
                nc.vector.tensor_mul(t_a[:], arr_p[:, 0:2], arr_p[:, 2:4])
                nc.vector.tensor_mul(t_b[:], arr_p[:, 0:2], arr_p[:, 0:2])
                nc.vector.tensor_mul(t_c[:], arr_p[:, 2:4], arr_p[:, 2:4])
                nc.vector.tensor_reduce(out=stack[:, 0:1], in_=t_a[:],
                                        axis=mybir.AxisListType.X,
                                        op=mybir.AluOpType.add)
                nc.vector.tensor_reduce(out=stack[:, 1:2], in_=t_b[:],
                                        axis=mybir.AxisListType.X,
                                        op=mybir.AluOpType.add)
                nc.vector.tensor_reduce(out=stack[:, 2:3], in_=t_c[:],
                                        axis=mybir.AxisListType.X,
                                        op=mybir.AluOpType.add)
                nc.vector.tensor_copy(out=stack[:, 3:4], in_=arr_w)
                stack_bf = lsp.tile([128, 4], dt.float32, tag="stackbf")
                nc.vector.tensor_copy(out=stack_bf[:], in_=stack[:])
                pfin = plp.tile([1, 4], dt.float32, space="PSUM", tag="pfin")
                ones_f = lsp.tile([128, 1], dt.float32, tag="onesf")
                nc.vector.memset(ones_f[:], 1.0)
                for jj in range(4):
                    nc.tensor.matmul(out=pfin[:, jj:jj + 1],
                                     lhsT=stack_bf[:, jj:jj + 1],
                                     rhs=ones_f[:], start=True, stop=True)
                # loss = 0.5*(1 - wcos/Mn) + 0.5*(1 - dot/sqrt(n1*n2))
                sc = lsp.tile([1, 4], dt.float32, tag="sc")
                nc.vector.tensor_copy(out=sc[:], in_=pfin[:])
                g2v = lsp.tile([1, 1], dt.float32, tag="g2v")
                nc.vector.tensor_mul(g2v[:], sc[:, 1:2], sc[:, 2:3])
                nc.vector.tensor_scalar_max(g2v[:], g2v[:], 1e-24)
                nc.vector.reciprocal(g2v[:], g2v[:])
                nc.scalar.activation(out=g2v[:], in_=g2v[:],
                                     func=mybir.ActivationFunctionType.Sqrt)
                nc.vector.tensor_mul(g2v[:], g2v[:], sc[:, 0:1])  # cos_g
                res = lsp.tile([1, 1], dt.float32, tag="res")
                nc.vector.tensor_scalar_mul(res[:], sc[:, 3:4],
                                            -0.5 / float(Mn))
                nc.vector.tensor_scalar_add(res[:], res[:], 1.0)
                half = lsp.tile([1, 1], dt.float32, tag="half")
                nc.vector.tensor_scalar_mul(half[:], g2v[:], 0.5)
                nc.vector.tensor_sub(res[:], res[:], half[:])
                nc.sync.dma_start(out=out_loss[:], in_=res[:])

    nc.compile()
    return nc


# ---------------- public entry ----------------
_CACHE = {}


def prepare(feat, mask_token, oW1, oW2, og1, ob1, og2, ob2,
            tW1, tW2, tg1, tb1, tg2, tb2, src, dst, mask_nodes):
    feat = np.asarray(feat, np.float32)
    mask_token = np.asarray(mask_token, np.float32)
    src = np.asarray(src, np.int32)
    dst = np.asarray(dst, np.int32)
    mask_nodes = np.asarray(mask_nodes, np.int32)

    sched = preprocess(src, dst)

    key = (sched[0]["T"], sched[1]["T"], len(mask_nodes))
    if key not in _CACHE:
        _CACHE[key] = build(sched, len(mask_nodes))
    nc = _CACHE[key]

    # host tensors
    x = feat.copy()
    x[mask_nodes] = np.broadcast_to(mask_token, (len(mask_nodes), D))
    xf = np.concatenate([x, feat], axis=1).astype(f8)   # [N, 256] fp8
    iota_c = np.broadcast_to(np.arange(WW, dtype=np.float32),
                             (128, WW)).astype(bf16)
    ones_c = np.ones((128, 1), bf16)
    ident_c = np.eye(128, dtype=np.float32).astype(bf16)
    is_m = np.bincount(mask_nodes, minlength=N).astype(np.float32)

    enc_params = [
        (oW1, oW2, og1, ob1, og2, ob2),   # e=0 online (masked input)
        (tW1, tW2, tg1, tb1, tg2, tb2),   # e=1 target
    ]

    common = {"xf_tab": xf, "iota_c": iota_c, "ones_c": ones_c,
              "ident_c": ident_c}
    for e in range(2):
        w1, w2, g1, b1, g2, b2 = enc_params[e]
        for l in range(2):
            common[f"w1_{e}{l}"] = np.asarray(w1[l], np.float32).astype(bf16)
            common[f"w2_{e}{l}"] = np.asarray(w2[l], np.float32).astype(bf16)
            common[f"g1_{e}{l}"] = np.asarray(g1[l], np.float32).reshape(2, 128).T.copy()
            common[f"b1_{e}{l}"] = np.asarray(b1[l], np.float32).reshape(2, 128).T.copy()
            common[f"g2_{e}{l}"] = np.asarray(g2[l], np.float32).reshape(128, 1).copy()
            common[f"b2_{e}{l}"] = np.asarray(b2[l], np.float32).reshape(128, 1).copy()

    in_maps = []
    for c in range(NCORES):
        rows = slice(c * NSHARD, (c + 1) * NSHARD)
        xT = np.zeros((128, NLOC), np.float32)
        fT = np.zeros((128, NLOC), np.float32)
        xT[:, :NSHARD] = x[rows].T
        fT[:, :NSHARD] = feat[rows].T
        wn = np.zeros(NLOC, np.float32)
        wn[:NSHARD] = is_m[rows]
        m = dict(common)
        m.update({
            "xT_own": xT.astype(bf16), "fT_own": fT.astype(bf16),
            "idx0": sched[0]["idx16"][c], "idx1": sched[1]["idx16"][c],
            "dst0": sched[0]["dstA"][c], "dst1": sched[1]["dstA"][c],
            "w_node": wn.reshape(NBLK, 128).T.copy(),
        })
        in_maps.append(m)

    return nc, in_maps


def kernel(**inputs):
    nc, in_maps = prepare(**inputs)
    last_err = None
    for attempt in range(3):
        try:
            res = run_bass_kernel_spmd(nc, in_maps, core_ids=list(range(NCORES)))
            loss = res.results[0]["loss"].reshape(())
            return np.float32(loss)
        except Exception as e:  # transient NRT device errors happen; retry
            last_err = e
    raise last_err


# revision 15
# speedup vs baseline: 1.4315x; 1.0369x over previous
"""GraphMAE-style 2-layer GIN loss kernel for one TRN2 chip (8 NeuronCores).

Self-contained: builds + compiles a Bass/Tile SPMD kernel, shards the graph on
the host (dst-partitioned nodes + incident edges), runs on 8 cores via
run_bass_kernel_spmd, returns the scalar loss.

Design (v2, optimized):
  - nodes sharded: core c owns rows [12500c, 12500(c+1)), padded to 12544
  - gather tables hold BOTH encoders' features per row (256B fp8):
    layer 0: [x | feat], layer 1: [h0 | h1]; one batched indirect DMA
    gathers GRP*128 edges per instruction (SWDGE cost is ~1us fixed +
    0.34ns/row, so batching is ~6x cheaper than per-tile gathers)
  - segment_sum: per dst-window of 256 nodes, PSUM accumulates
    gathered_rows.T @ onehot(dst); the node's own contribution is seeded
    into PSUM with an identity-weight matmul (z = own + agg)
  - one-hot built by DVE tensor_scalar is_equal (bf16, 4x perf mode)
  - BN1 stats: sum(y) = W1.T @ sum(z) (tiny matmul after AllReduce);
    sum(y^2) accumulated by ScalarE Square activations during edge phase
  - h tables written transposed (fp8) to a shared DRAM table via a
    4-chunk AllGather overlapped with the transpose writes
  - feat-major activations [128 feat, nodes]; BN affine + relu on ScalarE
"""
import sys, os, contextlib
sys.path.insert(0, '/opt/trn_rl_repo')
import numpy as np
import ml_dtypes

from concourse import bass, bacc, tile, mybir
from concourse.bass_utils import run_bass_kernel_spmd

dt = mybir.dt
bf16 = ml_dtypes.bfloat16
f8 = ml_dtypes.float8_e4m3

# ---------------- problem constants ----------------
N = 100000
E = 800000
D = 128
H = 256
L = 2
NCORES = 8
NSHARD = N // NCORES          # 12500
NLOC = 12544                  # padded to 98*128
WW = 256                      # dst-window width (nodes)
WIN = NLOC // WW              # 49 windows per core
NBLK = NLOC // 128            # 98 node blocks per core
BN_EPS = 1e-5
BLK_GRP = 4                   # node blocks per matmul group (free dim 512)
NGRP = (NBLK + BLK_GRP - 1) // BLK_GRP   # 25


# ---------------- host-side graph preprocessing ----------------
def preprocess(src, dst):
    """Build per-core edge streams + shared schedule, one per layer.

    Stream order: (window-pair, bin, window, src).  Each gather instruction
    covers the tiles of one (pair, bin) run; int16 indices are bin-local.
    """
    src = np.asarray(src).astype(np.int64)
    dst = np.asarray(dst).astype(np.int64)
    core_of = dst // NSHARD
    dst_loc = dst % NSHARD
    win = dst_loc // WW
    dloc = dst_loc % WW
    NPAIR = (WIN + 1) // 2

    out = []
    for layer in range(2):
        if layer == 0:
            row = src
            nrow = N
        else:
            row = (src // NSHARD) * NLOC + src % NSHARD
            nrow = NLOC * NCORES
        nbin = (nrow + 32767) // 32768
        bin_ = row // 32768
        loc16 = (row - bin_ * 32768).astype(np.int64)

        # per-core counts per (win, bin)
        counts = np.zeros((NCORES, WIN, nbin), np.int64)
        sels = []
        for c in range(NCORES):
            sel = np.nonzero(core_of == c)[0]
            order = np.lexsort((src[sel], bin_[sel], win[sel]))
            sel = sel[order]
            sels.append(sel)
            np.add.at(counts[c], (win[sel], bin_[sel]), 1)
        ntile = (counts.max(axis=0) + 127) // 128      # [WIN, nbin]

        # tile stream ordered (pair, bin, win); instruction per (pair, bin)
        win_of_tile = []
        instrs = []            # (t0, nt, bin)
        tile_start = np.zeros((WIN, nbin), np.int64)
        for pr in range(NPAIR):
            ws = [2 * pr] + ([2 * pr + 1] if 2 * pr + 1 < WIN else [])
            for b in range(nbin):
                nt = int(sum(ntile[w, b] for w in ws))
                if nt == 0:
                    continue
                t0 = len(win_of_tile)
                # dma_gather is limited to 1024 indices (8 tiles) per instr
                for o in range(0, nt, 8):
                    instrs.append((t0 + o, min(8, nt - o), b))
                for w in ws:
                    tile_start[w, b] = len(win_of_tile)
                    win_of_tile += [w] * int(ntile[w, b])
        T = len(win_of_tile)
        win_of_tile = np.array(win_of_tile, np.int64)

        idx16 = np.zeros((NCORES, 128, T * 8), np.int16)
        dstA = np.full((NCORES, T * 128), -1.0, np.float32)
        for c in range(NCORES):
            sel = sels[c]
            # rank within (win, bin) group
            ofs = np.concatenate([[0], np.cumsum(counts[c].ravel())])
            key = win[sel] * nbin + bin_[sel]
            pos = tile_start[win[sel], bin_[sel]] * 128 + \
                (np.arange(len(sel)) - ofs[key])
            # idx stream: edge at stream pos i -> [16*k + i%16, i//16]
            rows = (pos % 16).astype(np.int64)
            cols = (pos // 16).astype(np.int64)
            for k in range(8):
                idx16[c, 16 * k + rows, cols] = loc16[sel]
            dstA[c, pos] = dloc[sel]
        dstA = np.ascontiguousarray(
            dstA.reshape(NCORES, T, 128).transpose(0, 2, 1)).astype(np.float32)
        out.append(dict(T=T, win_of_tile=win_of_tile, instrs=instrs,
                        nbin=nbin, idx16=idx16, dstA=dstA))
    return out


# ---------------- kernel builder ----------------
def build(sched, Mn):
    nc = bacc.Bacc("TRN2", target_bir_lowering=False, debug=False,
                   num_devices=NCORES, num_swdge_queues=4)
    rg = [list(range(NCORES))]

    def inp(name, shape, d):
        return nc.dram_tensor(name, shape, d, kind="ExternalInput")

    xf_tab = inp("xf_tab", [N, 2 * D], dt.float8e4)
    xT_own = inp("xT_own", [128, NLOC], dt.bfloat16)
    fT_own = inp("fT_own", [128, NLOC], dt.bfloat16)
    idx_in = [inp(f"idx{l}", [128, sched[l]["T"] * 8], dt.int16)
              for l in range(2)]
    dst_in = [inp(f"dst{l}", [128, sched[l]["T"]], dt.float32)
              for l in range(2)]
    iota_c = inp("iota_c", [128, WW], dt.bfloat16)
    ones_c = inp("ones_c", [128, 1], dt.bfloat16)
    ident_c = inp("ident_c", [128, 128], dt.bfloat16)
    w_node = inp("w_node", [128, NBLK], dt.float32)
    W1 = {}; W2 = {}; G1 = {}; B1 = {}; G2 = {}; B2 = {}
    for e in range(2):
        for l in range(2):
            W1[e, l] = inp(f"w1_{e}{l}", [128, H], dt.bfloat16)
            W2[e, l] = inp(f"w2_{e}{l}", [H, 128], dt.bfloat16)
            G1[e, l] = inp(f"g1_{e}{l}", [128, 2], dt.float32)
            B1[e, l] = inp(f"b1_{e}{l}", [128, 2], dt.float32)
            G2[e, l] = inp(f"g2_{e}{l}", [128, 1], dt.float32)
            B2[e, l] = inp(f"b2_{e}{l}", [128, 1], dt.float32)
    out_loss = nc.dram_tensor("loss", [1, 1], dt.float32, kind="ExternalOutput")
    DEBUG = bool(int(os.environ.get("GIN_DEBUG", "0")))
    dbg = {}
    if DEBUG:
        for nm in ["zt0", "zt1", "h0", "h1", "zl0", "zl1", "hf0", "hf1"]:
            dbg[nm] = nc.dram_tensor(f"dbg_{nm}", [128, NLOC], dt.bfloat16,
                                     kind="ExternalOutput")
        dbg["st"] = nc.dram_tensor("dbg_st", [128, 8], dt.float32,
                                   kind="ExternalOutput")
        dbg["pools"] = nc.dram_tensor("dbg_pools", [128, 5], dt.float32,
                                      kind="ExternalOutput")

    with tile.TileContext(nc) as tc:
        with tc.tile_pool(name="const", bufs=1) as cst, \
             tc.tile_pool(name="streams", bufs=1) as stp, \
             tc.tile_pool(name="big", bufs=1) as bigp, \
             tc.tile_pool(name="gath", bufs=3) as gp, \
             tc.tile_pool(name="sti", bufs=12) as sti, \
             tc.tile_pool(name="small", bufs=4) as smp, \
             tc.tile_pool(name="stats", bufs=1) as statp, \
             tc.tile_pool(name="dram", bufs=2, space="DRAM") as dramp:

            htab_loc = dramp.tile([NLOC, 2 * D], dt.float8e4,
                                  tag="htab_loc", name="htab_loc")
            htab = dramp.tile([NLOC * NCORES, 2 * D], dt.float8e4, tag="htab",
                              name="htab", addr_space="Shared")

            iota_t = cst.tile([128, WW], dt.bfloat16)
            ones_t = cst.tile([128, 1], dt.bfloat16)
            ident_t = cst.tile([128, 128], dt.bfloat16)
            wnode_t = cst.tile([128, NBLK], dt.float32)
            nc.sync.dma_start(out=iota_t[:], in_=iota_c[:])
            nc.sync.dma_start(out=ones_t[:], in_=ones_c[:])
            nc.sync.dma_start(out=ident_t[:], in_=ident_c[:])
            nc.sync.dma_start(out=wnode_t[:], in_=w_node[:])
            wt = {}
            for e in range(2):
                for l in range(2):
                    wt[e, l] = dict(
                        w1=cst.tile([128, H], dt.bfloat16, tag=f"w1_{e}{l}",
                                    name=f"w1t{e}{l}"),
                        w2a=cst.tile([128, 128], dt.bfloat16, tag=f"w2a_{e}{l}",
                                     name=f"w2at{e}{l}"),
                        w2b=cst.tile([128, 128], dt.bfloat16, tag=f"w2b_{e}{l}",
                                     name=f"w2bt{e}{l}"),
                        g1=cst.tile([128, 2], dt.float32, tag=f"g1_{e}{l}",
                                    name=f"g1t{e}{l}"),
                        b1=cst.tile([128, 2], dt.float32, tag=f"b1_{e}{l}",
                                    name=f"b1t{e}{l}"),
                        g2=cst.tile([128, 1], dt.float32, tag=f"g2_{e}{l}",
                                    name=f"g2t{e}{l}"),
                        b2=cst.tile([128, 1], dt.float32, tag=f"b2_{e}{l}",
                                    name=f"b2t{e}{l}"),
                    )
                    nc.sync.dma_start(out=wt[e, l]["w1"][:], in_=W1[e, l][:])
                    nc.sync.dma_start(out=wt[e, l]["w2a"][:], in_=W2[e, l][:128, :])
                    nc.sync.dma_start(out=wt[e, l]["w2b"][:], in_=W2[e, l][128:, :])
                    nc.sync.dma_start(out=wt[e, l]["g1"][:], in_=G1[e, l][:])
                    nc.sync.dma_start(out=wt[e, l]["b1"][:], in_=B1[e, l][:])
                    nc.sync.dma_start(out=wt[e, l]["g2"][:], in_=G2[e, l][:])
                    nc.sync.dma_start(out=wt[e, l]["b2"][:], in_=B2[e, l][:])

            # big activations
            zT = [bigp.tile([128, NLOC], dt.bfloat16, tag=f"zT{e}",
                            name=f"zT{e}") for e in range(2)]
            hT = [bigp.tile([128, NLOC], dt.bfloat16, tag=f"hT{e}",
                            name=f"hTe{e}") for e in range(2)]
            z2T = [None, None]
            # per-(layer,enc) BN1 partial stats filled during the edge phase
            fusedA = {l: {"ysq": {}, "zs": {}, "arr": {}} for l in range(2)}
            for l in range(2):
                for e in range(2):
                    fusedA[l]["ysq"][e] = statp.tile(
                        [128, 2 * NGRP], dt.float32,
                        tag=f"fysq{l}{e}", name=f"fysq{l}{e}")
                    fusedA[l]["zs"][e] = statp.tile(
                        [128, WIN], dt.float32,
                        tag=f"fzs{l}{e}", name=f"fzs{l}{e}")
            pools = statp.tile([128, 4], dt.float32)  # cols: e0l0,e0l1,e1l0,e1l1
            wcos_acc = statp.tile([128, 1], dt.float32)
            nc.vector.memset(pools[:], 0.0)
            nc.vector.memset(wcos_acc[:], 0.0)

            def all_reduce_stats(stats_tile, ncols):
                """DMA stats [128, ncols] -> AR -> return SBUF tile with result."""
                ari = dramp.tile([128, 8], dt.float32, tag="arin")
                aro = dramp.tile([128, 8], dt.float32, tag="arout",
                                 addr_space="Shared")
                nc.sync.dma_start(out=ari[:, :ncols], in_=stats_tile[:, :ncols])
                if ncols < 8:
                    zpad = smp.tile([128, 8], dt.float32, tag="zpad")
                    nc.vector.memset(zpad[:], 0.0)
                    nc.sync.dma_start(out=ari[:, ncols:], in_=zpad[:, ncols:])
                nc.gpsimd.collective_compute(
                    "AllReduce", mybir.AluOpType.add, replica_groups=rg,
                    ins=[ari[:].opt()], outs=[aro[:].opt()])
                res = smp.tile([128, 8], dt.float32, tag="arres")
                nc.sync.dma_start(out=res[:, :ncols], in_=aro[:, :ncols])
                return res

            # ---------------- edge phase ----------------
            def edge_phase(layer):
                """Produces zT[0], zT[1] = own + agg for both encoders."""
                ctx = nc.named_scope(f"edge{layer}"); ctx.__enter__()
                sc = sched[layer]
                T = sc["T"]
                win_of_tile = sc["win_of_tile"]
                instrs = sc["instrs"]
                table = xf_tab if layer == 0 else htab
                nrow = N if layer == 0 else NLOC * NCORES
                maxnt = max(nt for _, nt, _ in instrs)
                first_of_win = {}
                last_of_win = {}
                tile_instr = {}
                for gi, (t0, nt, b) in enumerate(instrs):
                    for t in range(t0, t0 + nt):
                        tile_instr[t] = gi
                for t in range(T):
                    w = int(win_of_tile[t])
                    first_of_win.setdefault(w, t)
                    last_of_win[w] = t
                gbufs = {}
                with tc.tile_pool(name=f"psum_e{layer}", bufs=1, space="PSUM") as pp, \
                     tc.tile_pool(name=f"psum_a{layer}", bufs=1, space="PSUM") as ppA, \
                     tc.tile_pool(name=f"estr{layer}", bufs=1) as estp, \
                     tc.tile_pool(name=f"gath{layer}", bufs=1) as gp, \
                     tc.tile_pool(name=f"own{layer}", bufs=4) as ownp:
                    idx_sb = estp.tile([128, T * 8], dt.int16, tag="idx")
                    nc.sync.dma_start(out=idx_sb[:], in_=idx_in[layer][:])
                    dst_sb = estp.tile([128, T], dt.float32, tag="dst")
                    nc.sync.dma_start(out=dst_sb[:], in_=dst_in[layer][:])

                    def issue_gather(gi):
                        t0, nt, b = instrs[gi]
                        b1 = min((b + 1) * 32768, nrow)
                        gb = gp.tile([128, maxnt * 2 * D], dt.float8e4,
                                     tag=f"g{gi % 8}", name=f"g{gi % 8}")
                        nc.gpsimd.dma_gather(
                            out_ap=gb[:, :nt * 2 * D].rearrange(
                                "p (t f) -> p t f", f=2 * D),
                            in_ap=table[b * 32768:b1, :],
                            idxs_ap=idx_sb[:, t0 * 8:(t0 + nt) * 8],
                            num_idxs=nt * 128,
                            num_idxs_reg=nt * 128,
                            elem_size=2 * D,
                            queue_num=gi % 4)
                        gbufs[gi] = gb

                    owns = {}

                    def issue_own(w):
                        cols = slice(w * WW, (w + 1) * WW)
                        if layer == 0:
                            o1 = ownp.tile([128, WW], dt.bfloat16, tag="own1")
                            o2 = ownp.tile([128, WW], dt.bfloat16, tag="own2")
                            nc.sync.dma_start(out=o1[:], in_=xT_own[:, cols])
                            nc.sync.dma_start(out=o2[:], in_=fT_own[:, cols])
                            owns[w] = (o1[:], o2[:])
                        else:
                            owns[w] = (hT[0][:, cols], hT[1][:, cols])

                    def passA_group(gidx):
                        gsz = min(BLK_GRP, NBLK - gidx * BLK_GRP)
                        ncols = gsz * 128
                        c0 = gidx * BLK_GRP * 128
                        for e in (1, 0):
                            p = wt[e, layer]
                            for h in range(2):
                                psy = ppA.tile([128, 512], dt.float32,
                                               space="PSUM", tag=f"psyA{h}",
                                               name=f"psyA{h}")
                                nc.tensor.matmul(
                                    out=psy[:, :ncols],
                                    lhsT=p["w1"][:, h * 128:(h + 1) * 128],
                                    rhs=zT[e][:, c0:c0 + ncols],
                                    start=True, stop=True)
                                deadA = smp.tile([128, 512], dt.bfloat16,
                                                 tag="deadA", name="deadA")
                                nc.scalar.activation(
                                    out=deadA[:, :ncols], in_=psy[:, :ncols],
                                    func=mybir.ActivationFunctionType.Square,
                                    accum_out=fusedA[layer]["ysq"][e][
                                        :, 2 * gidx + h:2 * gidx + h + 1])

                    issue_own(0)
                    next_g = 0
                    psum = {}
                    for t in range(T):
                        gi = tile_instr[t]
                        while next_g <= min(gi + 6, len(instrs) - 1):
                            issue_gather(next_g)
                            next_g += 1
                        w = int(win_of_tile[t])
                        first = (t == first_of_win[w])
                        last = (t == last_of_win[w])
                        if first:
                            if w + 1 < WIN:
                                issue_own(w + 1)
                            psA = pp.tile([128, WW], dt.float32, space="PSUM",
                                          tag=f"psA{w % 3}")
                            psB = pp.tile([128, WW], dt.float32, space="PSUM",
                                          tag=f"psB{w % 3}")
                            psum[w] = (psA, psB)
                            o1, o2 = owns.pop(w)
                            nc.tensor.matmul(out=psA[:], lhsT=ident_t[:],
                                             rhs=o1, start=True, stop=False)
                            nc.tensor.matmul(out=psB[:], lhsT=ident_t[:],
                                             rhs=o2, start=True, stop=False)
                        psA, psB = psum[w]
                        Sf = sti.tile([128, WW], dt.bfloat16, tag="Sf",
                                      name="Sf")
                        nc.vector.tensor_scalar(
                            out=Sf[:], in0=iota_t[:],
                            scalar1=dst_sb[:, t:t + 1], scalar2=None,
                            op0=mybir.AluOpType.is_equal)
                        gb = gbufs[gi]
                        j = t - instrs[gi][0]
                        nc.tensor.matmul(out=psA[:],
                                         lhsT=gb[:, j * 2 * D:j * 2 * D + D],
                                         rhs=Sf[:], start=False, stop=last)
                        nc.tensor.matmul(out=psB[:],
                                         lhsT=gb[:, j * 2 * D + D:(j + 1) * 2 * D],
                                         rhs=Sf[:], start=False, stop=last)
                        if last:
                            del psum[w]
                            cols = slice(w * WW, (w + 1) * WW)
                            nc.scalar.activation(
                                out=zT[0][:, cols], in_=psA[:],
                                func=mybir.ActivationFunctionType.Copy,
                                accum_out=fusedA[layer]["zs"][0][:, w:w + 1])
                            nc.scalar.activation(
                                out=zT[1][:, cols], in_=psB[:],
                                func=mybir.ActivationFunctionType.Copy,
                                accum_out=fusedA[layer]["zs"][1][:, w:w + 1])
                            if w % 2 == 1 or w == WIN - 1:
                                passA_group(w // 2)
                    # final stat reduces + AR issue for both encoders
                    for e in (1, 0):
                        st = statp.tile([128, 4], dt.float32, tag="bnstat",
                                        name="bnstat")
                        for h in range(2):
                            nc.vector.tensor_reduce(
                                out=st[:, h:h + 1],
                                in_=fusedA[layer]["ysq"][e][:, h:2 * NGRP:2],
                                axis=mybir.AxisListType.X,
                                op=mybir.AluOpType.add)
                        nc.vector.tensor_reduce(
                            out=st[:, 2:3], in_=fusedA[layer]["zs"][e][:],
                            axis=mybir.AxisListType.X, op=mybir.AluOpType.add)
                        fusedA[layer]["arr"][e] = all_reduce_stats(st, 3)

                ctx.__exit__(None, None, None)

            # ---------------- node phase ----------------
            def node_phase(e, layer, write_table, pp, nsp, trp, ptp):
                p = wt[e, layer]
                z = zT[e]
                with contextlib.nullcontext():
                    arr = fusedA[layer]["arr"][e]
                    yield
                    # BN1: mean via W1.T @ zsum (tiny matmuls post-AR)
                    zsum_bf = smp.tile([128, 1], dt.bfloat16, tag="zsbf")
                    nc.vector.tensor_copy(out=zsum_bf[:], in_=arr[:, 2:3])
                    psyS = pp.tile([128, 512], dt.float32, space="PSUM",
                                   tag="psz")
                    for h in range(2):
                        nc.tensor.matmul(out=psyS[:, h:h + 1],
                                         lhsT=p["w1"][:, h * 128:(h + 1) * 128],
                                         rhs=zsum_bf[:], start=True, stop=True)
                    ysums = smp.tile([128, 2], dt.float32, tag="ysums")
                    nc.vector.tensor_copy(out=ysums[:], in_=psyS[:, 0:2])
                    # affine: s = g/sqrt(var+eps), t = b - mean*s  (per half)
                    s1 = smp.tile([128, 2], dt.float32, tag="s1")
                    t1 = smp.tile([128, 2], dt.float32, tag="t1")
                    mean = smp.tile([128, 2], dt.float32, tag="mean")
                    var = smp.tile([128, 2], dt.float32, tag="var")
                    nc.vector.tensor_scalar_mul(mean[:], ysums[:], 1.0 / N)
                    nc.vector.tensor_scalar_mul(var[:], arr[:, 0:2], 1.0 / N)
                    msq = smp.tile([128, 2], dt.float32, tag="msq")
                    nc.vector.tensor_mul(msq[:], mean[:], mean[:])
                    nc.vector.tensor_sub(var[:], var[:], msq[:])
                    nc.vector.tensor_scalar_add(var[:], var[:], BN_EPS)
                    nc.vector.reciprocal(var[:], var[:])
                    nc.scalar.activation(out=var[:], in_=var[:],
                                         func=mybir.ActivationFunctionType.Sqrt)
                    nc.vector.tensor_mul(s1[:], p["g1"][:], var[:])
                    nc.vector.tensor_mul(t1[:], mean[:], s1[:])
                    nc.vector.tensor_sub(t1[:], p["b1"][:], t1[:])

                    # ---- pass B: recompute y, BN1+relu, W2 -> z2, stats2 ----
                    z2sum = nsp.tile([128, NGRP + 1], dt.float32, tag="z2sum")
                    z2sq = nsp.tile([128, NGRP + 1], dt.float32, tag="z2sq")
                    for g in range(NGRP):
                        gsz = min(BLK_GRP, NBLK - g * BLK_GRP)
                        ncols = gsz * 128
                        c0 = g * BLK_GRP * 128
                        zsl = z[:, c0:c0 + ncols]
                        psz = pp.tile([128, 512], dt.float32, space="PSUM",
                                      tag="psz")
                        for h in range(2):
                            psy = pp.tile([128, 512], dt.float32, space="PSUM",
                                          tag=f"psy{h}")
                            nc.tensor.matmul(out=psy[:, :ncols],
                                             lhsT=p["w1"][:, h * 128:(h + 1) * 128],
                                             rhs=zsl, start=True, stop=True)
                            hm = nsp.tile([128, 512], dt.bfloat16, tag=f"hm{h}")
                            nc.scalar.activation(
                                out=hm[:, :ncols], in_=psy[:, :ncols],
                                func=mybir.ActivationFunctionType.Relu,
                                bias=t1[:, h:h + 1], scale=s1[:, h:h + 1])
                            nc.tensor.matmul(out=psz[:, :ncols],
                                             lhsT=p["w2a" if h == 0 else "w2b"][:],
                                             rhs=hm[:, :ncols],
                                             start=(h == 0), stop=(h == 1))
                        nc.vector.tensor_copy(out=z2T[e][:, c0:c0 + ncols],
                                              in_=psz[:, :ncols])
                        # exclude padded phantom nodes from stats
                        vcols = min(ncols, NSHARD - c0)
                        dead = nsp.tile([128, 512], dt.bfloat16, tag="dead")
                        nc.scalar.activation(
                            out=dead[:, :vcols], in_=psz[:, :vcols],
                            func=mybir.ActivationFunctionType.Square,
                            accum_out=z2sq[:, g:g + 1])
                        nc.vector.tensor_reduce(
                            out=z2sum[:, g:g + 1], in_=psz[:, :vcols],
                            axis=mybir.AxisListType.X, op=mybir.AluOpType.add)
                    if NLOC > NSHARD:
                        nc.vector.memset(z2T[e][:, NSHARD:NLOC], 0.0)
                    st2 = statp.tile([128, 4], dt.float32, tag="bnstat2")
                    nc.vector.tensor_reduce(out=st2[:, 0:1], in_=z2sq[:, :NGRP],
                                            axis=mybir.AxisListType.X,
                                            op=mybir.AluOpType.add)
                    nc.vector.tensor_reduce(out=st2[:, 1:2], in_=z2sum[:, :NGRP],
                                            axis=mybir.AxisListType.X,
                                            op=mybir.AluOpType.add)
                    arr2 = all_reduce_stats(st2, 2)
                    yield
                    s2 = smp.tile([128, 1], dt.float32, tag="s2")
                    t2 = smp.tile([128, 1], dt.float32, tag="t2")
                    mean2 = smp.tile([128, 1], dt.float32, tag="mean2")
                    var2 = smp.tile([128, 1], dt.float32, tag="var2")
                    nc.vector.tensor_scalar_mul(mean2[:], arr2[:, 1:2], 1.0 / N)
                    nc.vector.tensor_scalar_mul(var2[:], arr2[:, 0:1], 1.0 / N)
                    msq2 = smp.tile([128, 1], dt.float32, tag="msq2")
                    nc.vector.tensor_mul(msq2[:], mean2[:], mean2[:])
                    nc.vector.tensor_sub(var2[:], var2[:], msq2[:])
                    nc.vector.tensor_scalar_add(var2[:], var2[:], BN_EPS)
                    nc.vector.reciprocal(var2[:], var2[:])
                    nc.scalar.activation(out=var2[:], in_=var2[:],
                                         func=mybir.ActivationFunctionType.Sqrt)
                    nc.vector.tensor_mul(s2[:], p["g2"][:], var2[:])
                    nc.vector.tensor_mul(t2[:], mean2[:], s2[:])
                    nc.vector.tensor_sub(t2[:], p["b2"][:], t2[:])

                    # ---- pass C: h = relu(BN2(z2)), pools, optional table ----
                    poolstage = nsp.tile([128, NGRP + 1], dt.float32, tag="pst")

                    def passC_groups(g0_, g1_):
                        for g in range(g0_, g1_):
                            gsz = min(BLK_GRP, NBLK - g * BLK_GRP)
                            ncols = gsz * 128
                            c0 = g * BLK_GRP * 128
                            nc.scalar.activation(
                                out=hT[e][:, c0:c0 + ncols],
                                in_=z2T[e][:, c0:c0 + ncols],
                                func=mybir.ActivationFunctionType.Relu,
                                bias=t2[:, 0:1], scale=s2[:, 0:1],
                                accum_out=poolstage[:, g:g + 1])
                            if write_table:
                                stg = trp.tile([128, 512], dt.float8e4,
                                               tag="stg")
                                for jj in range(gsz):
                                    blk = g * BLK_GRP + jj
                                    pst = ptp.tile([128, 128], dt.bfloat16,
                                                   space="PSUM", tag="pt")
                                    nc.tensor.transpose(
                                        out=pst[:],
                                        in_=hT[e][:, blk * 128:(blk + 1) * 128],
                                        identity=ident_t[:])
                                    nc.vector.tensor_copy(
                                        out=stg[:, jj * 128:(jj + 1) * 128],
                                        in_=pst[:])
                                nc.sync.dma_start(
                                    out=htab_loc[
                                        g * 512:g * 512 + gsz * 128,
                                        e * D:(e + 1) * D].rearrange(
                                        "(c p) f -> p c f", p=128),
                                    in_=stg[:, :gsz * 128].rearrange(
                                        "p (c f) -> p c f", f=128))

                    passC_groups(0, NGRP)

                    # pools: pad correction (z2 pad cols are 0 -> h=relu(t2))
                    npad = NLOC - NSHARD
                    relut2 = smp.tile([128, 1], dt.float32, tag="relut2")
                    nc.vector.tensor_scalar_max(relut2[:], t2[:], 0.0)
                    nc.vector.tensor_scalar_mul(relut2[:], relut2[:],
                                                -float(npad))
                    pcol = pools[:, 2 * e + layer:2 * e + layer + 1]
                    nc.vector.tensor_reduce(out=pcol, in_=poolstage[:, :NGRP],
                                            axis=mybir.AxisListType.X,
                                            op=mybir.AluOpType.add)
                    nc.vector.tensor_add(out=pcol, in0=pcol, in1=relut2[:])
                    if NLOC > NSHARD:
                        nc.vector.memset(hT[e][:, NSHARD:NLOC], 0.0)

            def run_node_layer(layer, write_table):
                with tc.tile_pool(name=f"psum_n{layer}", bufs=2,
                                  space="PSUM") as pp, \
                     tc.tile_pool(name=f"nstage{layer}", bufs=3) as nsp, \
                     tc.tile_pool(name=f"zz{layer}", bufs=1) as zzp, \
                     tc.tile_pool(name=f"tr{layer}", bufs=3) as trp, \
                     tc.tile_pool(name=f"ptr{layer}", bufs=2,
                                  space="PSUM") as ptp:
                    for e in range(2):
                        z2T[e] = zzp.tile([128, NLOC], dt.bfloat16,
                                          tag=f"z2T{e}", name=f"z2T{e}_{layer}")
                    g1 = node_phase(1, layer, write_table, pp, nsp, trp, ptp)
                    g0 = node_phase(0, layer, write_table, pp, nsp, trp, ptp)
                    next(g1); next(g0)
                    next(g1); next(g0)
                    for _ in g1:
                        pass
                    for _ in g0:
                        pass
                    if write_table:
                        nc.gpsimd.collective_compute(
                            "AllGather", mybir.AluOpType.bypass,
                            replica_groups=rg,
                            ins=[htab_loc[:].opt()], outs=[htab[:].opt()])

            # ---------------- run the pipeline ----------------
            edge_phase(0)
            if DEBUG:
                nc.sync.dma_start(out=dbg["zt0"][:], in_=zT[0][:])
                nc.sync.dma_start(out=dbg["zt1"][:], in_=zT[1][:])
            run_node_layer(0, write_table=True)
            if DEBUG:
                nc.sync.dma_start(out=dbg["h0"][:], in_=hT[0][:])
                nc.sync.dma_start(out=dbg["h1"][:], in_=hT[1][:])
            edge_phase(1)
            if DEBUG:
                nc.sync.dma_start(out=dbg["zl0"][:], in_=zT[0][:])
                nc.sync.dma_start(out=dbg["zl1"][:], in_=zT[1][:])
            run_node_layer(1, write_table=False)
            if DEBUG:
                nc.sync.dma_start(out=dbg["hf0"][:], in_=hT[0][:])
                nc.sync.dma_start(out=dbg["hf1"][:], in_=hT[1][:])

            # ---------------- loss ----------------
            with tc.tile_pool(name="psum_l", bufs=2, space="PSUM") as plp, \
                 tc.tile_pool(name="lstage", bufs=3) as lsp:
                A = lsp.tile([128, NBLK], dt.float32, tag="A")
                B = lsp.tile([128, NBLK], dt.float32, tag="B")
                C = lsp.tile([128, NBLK], dt.float32, tag="C")
                for g in range(NGRP):
                    gsz = min(BLK_GRP, NBLK - g * BLK_GRP)
                    ncols = gsz * 128
                    c0 = g * BLK_GRP * 128
                    u = lsp.tile([128, 512], dt.bfloat16, tag="u")
                    q1 = lsp.tile([128, 512], dt.bfloat16, tag="q1")
                    q2 = lsp.tile([128, 512], dt.bfloat16, tag="q2")
                    nc.vector.tensor_mul(u[:, :ncols], hT[0][:, c0:c0 + ncols],
                                         hT[1][:, c0:c0 + ncols])
                    nc.vector.tensor_mul(q1[:, :ncols], hT[0][:, c0:c0 + ncols],
                                         hT[0][:, c0:c0 + ncols])
                    nc.vector.tensor_mul(q2[:, :ncols], hT[1][:, c0:c0 + ncols],
                                         hT[1][:, c0:c0 + ncols])
                    pa = plp.tile([128, BLK_GRP], dt.float32, space="PSUM",
                                  tag="pa")
                    pb = plp.tile([128, BLK_GRP], dt.float32, space="PSUM",
                                  tag="pb")
                    pc2 = plp.tile([128, BLK_GRP], dt.float32, space="PSUM",
                                   tag="pc2")
                    for jj in range(gsz):
                        sl = slice(jj * 128, (jj + 1) * 128)
                        nc.tensor.matmul(out=pa[:, jj:jj + 1], lhsT=u[:, sl],
                                         rhs=ones_t[:], start=True, stop=True)
                        nc.tensor.matmul(out=pb[:, jj:jj + 1], lhsT=q1[:, sl],
                                         rhs=ones_t[:], start=True, stop=True)
                        nc.tensor.matmul(out=pc2[:, jj:jj + 1], lhsT=q2[:, sl],
                                         rhs=ones_t[:], start=True, stop=True)
                    gs = slice(g * BLK_GRP, g * BLK_GRP + gsz)
                    nc.vector.tensor_copy(out=A[:, gs], in_=pa[:, :gsz])
                    nc.vector.tensor_copy(out=B[:, gs], in_=pb[:, :gsz])
                    nc.vector.tensor_copy(out=C[:, gs], in_=pc2[:, :gsz])
                # wcos = w * a / sqrt(b*c)
                BC = lsp.tile([128, NBLK], dt.float32, tag="BC")
                nc.vector.tensor_mul(BC[:], B[:], C[:])
                nc.vector.tensor_scalar_max(BC[:], BC[:], 1e-24)
                nc.vector.reciprocal(BC[:], BC[:])
                nc.scalar.activation(out=BC[:], in_=BC[:],
                                     func=mybir.ActivationFunctionType.Sqrt)
                nc.vector.tensor_mul(BC[:], BC[:], A[:])
                nc.vector.tensor_mul(BC[:], BC[:], wnode_t[:])
                nc.vector.tensor_reduce(out=wcos_acc[:], in_=BC[:],
                                        axis=mybir.AxisListType.X,
                                        op=mybir.AluOpType.add)

                if DEBUG:
                    nc.sync.dma_start(out=dbg["pools"][:, 0:4], in_=pools[:])
                    nc.sync.dma_start(out=dbg["pools"][:, 4:5], in_=wcos_acc[:])
                fin = statp.tile([128, 5], dt.float32, tag="fin")
                nc.vector.tensor_copy(out=fin[:, 0:4], in_=pools[:])
                nc.vector.tensor_copy(out=fin[:, 4:5], in_=wcos_acc[:])
                arr_p = all_reduce_stats(fin, 5)
                arr_w = arr_p[:, 4:5]
                stack = lsp.tile([128, 4], dt.float32, tag="stack")
                t_a = lsp.tile([128, 2], dt.float32, tag="t_a")
                t_b = lsp.tile([128, 2], dt.float32, tag="t_b")
                t_c = lsp.tile([128, 2], dt.float32, tag="t_c")
                nc.vector.tensor_mul(t_a[:], arr_p[:, 0:2], arr_p[:, 2:4])
                nc.vector.tensor_mul(t_b[:], arr_p[:, 0:2], arr_p[:, 0:2])
                nc.vector.tensor_mul(t_c[:], arr_p[:, 2:4], arr_p[:, 2:4])
                nc.vector.tensor_reduce(out=stack[:, 0:1], in_=t_a[:],
                                        axis=mybir.AxisListType.X,
                                        op=mybir.AluOpType.add)
                nc.vector.tensor_reduce(out=stack[:, 1:2], in_=t_b[:],
                                        axis=mybir.AxisListType.X,
                                        op=mybir.AluOpType.add)
                nc.vector.tensor_reduce(out=stack[:, 2:3], in_=t_c[:],
                                        axis=mybir.AxisListType.X,
                                        op=mybir.AluOpType.add)
                nc.vector.tensor_copy(out=stack[:, 3:4], in_=arr_w)
                stack_bf = lsp.tile([128, 4], dt.float32, tag="stackbf")
                nc.vector.tensor_copy(out=stack_bf[:], in_=stack[:])
                pfin = plp.tile([1, 4], dt.float32, space="PSUM", tag="pfin")
                ones_f = lsp.tile([128, 1], dt.float32, tag="onesf")
                nc.vector.memset(ones_f[:], 1.0)
                for jj in range(4):
                    nc.tensor.matmul(out=pfin[:, jj:jj + 1],
                                     lhsT=stack_bf[:, jj:jj + 1],
                                     rhs=ones_f[:], start=True, stop=True)
                # loss = 0.5*(1 - wcos/Mn) + 0.5*(1 - dot/sqrt(n1*n2))
                sc = lsp.tile([1, 4], dt.float32, tag="sc")
                nc.vector.tensor_copy(out=sc[:], in_=pfin[:])
                g2v = lsp.tile([1, 1], dt.float32, tag="g2v")
                nc.vector.tensor_mul(g2v[:], sc[:, 1:2], sc[:, 2:3])
                nc.vector.tensor_scalar_max(g2v[:], g2v[:], 1e-24)
                nc.vector.reciprocal(g2v[:], g2v[:])
                nc.scalar.activation(out=g2v[:], in_=g2v[:],
                                     func=mybir.ActivationFunctionType.Sqrt)
                nc.vector.tensor_mul(g2v[:], g2v[:], sc[:, 0:1])  # cos_g
                res = lsp.tile([1, 1], dt.float32, tag="res")
                nc.vector.tensor_scalar_mul(res[:], sc[:, 3:4],
                                            -0.5 / float(Mn))
                nc.vector.tensor_scalar_add(res[:], res[:], 1.0)
                half = lsp.tile([1, 1], dt.float32, tag="half")
                nc.vector.tensor_scalar_mul(half[:], g2v[:], 0.5)
                nc.vector.tensor_sub(res[:], res[:], half[:])
                nc.sync.dma_start(out=out_loss[:], in_=res[:])

    nc.compile()
    return nc


# ---------------- public entry ----------------
_CACHE = {}


def prepare(feat, mask_token, oW1, oW2, og1, ob1, og2, ob2,
            tW1, tW2, tg1, tb1, tg2, tb2, src, dst, mask_nodes):
    feat = np.asarray(feat, np.float32)
    mask_token = np.asarray(mask_token, np.float32)
    src = np.asarray(src, np.int32)
    dst = np.asarray(dst, np.int32)
    mask_nodes = np.asarray(mask_nodes, np.int32)

    sched = preprocess(src, dst)

    key = (sched[0]["T"], sched[1]["T"], len(mask_nodes))
    if key not in _CACHE:
        _CACHE[key] = build(sched, len(mask_nodes))
    nc = _CACHE[key]

    # host tensors
    x = feat.copy()
    x[mask_nodes] = np.broadcast_to(mask_token, (len(mask_nodes), D))
    xf = np.concatenate([x, feat], axis=1).astype(f8)   # [N, 256] fp8
    iota_c = np.broadcast_to(np.arange(WW, dtype=np.float32),
                             (128, WW)).astype(bf16)
    ones_c = np.ones((128, 1), bf16)
    ident_c = np.eye(128, dtype=np.float32).astype(bf16)
    is_m = np.bincount(mask_nodes, minlength=N).astype(np.float32)

    enc_params = [
        (oW1, oW2, og1, ob1, og2, ob2),   # e=0 online (masked input)
        (tW1, tW2, tg1, tb1, tg2, tb2),   # e=1 target
    ]

    common = {"xf_tab": xf, "iota_c": iota_c, "ones_c": ones_c,
              "ident_c": ident_c}
    for e in range(2):
        w1, w2, g1, b1, g2, b2 = enc_params[e]
        for l in range(2):
            common[f"w1_{e}{l}"] = np.asarray(w1[l], np.float32).astype(bf16)
            common[f"w2_{e}{l}"] = np.asarray(w2[l], np.float32).astype(bf16)
            common[f"g1_{e}{l}"] = np.asarray(g1[l], np.float32).reshape(2, 128).T.copy()
            common[f"b1_{e}{l}"] = np.asarray(b1[l], np.float32).reshape(2, 128).T.copy()
            common[f"g2_{e}{l}"] = np.asarray(g2[l], np.float32).reshape(128, 1).copy()
            common[f"b2_{e}{l}"] = np.asarray(b2[l], np.float32).reshape(128, 1).copy()

    in_maps = []
    for c in range(NCORES):
        rows = slice(c * NSHARD, (c + 1) * NSHARD)
        xT = np.zeros((128, NLOC), np.float32)
        fT = np.zeros((128, NLOC), np.float32)
        xT[:, :NSHARD] = x[rows].T
        fT[:, :NSHARD] = feat[rows].T
        wn = np.zeros(NLOC, np.float32)
        wn[:NSHARD] = is_m[rows]
        m = dict(common)
        m.update({
            "xT_own": xT.astype(bf16), "fT_own": fT.astype(bf16),
            "idx0": sched[0]["idx16"][c], "idx1": sched[1]["idx16"][c],
            "dst0": sched[0]["dstA"][c], "dst1": sched[1]["dstA"][c],
            "w_node": wn.reshape(NBLK, 128).T.copy(),
        })
        in_maps.append(m)

    return nc, in_maps


def kernel(**inputs):
    nc, in_maps = prepare(**inputs)
    last_err = None
    for attempt in range(3):
        try:
            res = run_bass_kernel_spmd(nc, in_maps, core_ids=list(range(NCORES)))
            loss = res.results[0]["loss"].reshape(())
            return np.float32(loss)
        except Exception as e:  # transient NRT device errors happen; retry
            last_err = e
    raise last_err
